# revision 1
# baseline (speedup 1.0000x reference)
"""Trainium2 Bass kernel for nn_Cp_linear_2D (CP/low-rank linear layer).

reference: W = einsum("ir,r,or->io", U1, lam, U2); y = x @ W + bias
  x: [4096, 4096], U1: [4096, 256], U2: [4096, 256], lam: [256], bias: [4096]

Strategy (8 cores, data-parallel over batch):
  - Never materialize W. Factored form: y = ((x @ U1) * lam) @ U2.T + bias
    (17 GFLOP instead of 154 GFLOP).
  - Each core gets a 512-row batch shard of x, pre-transposed on host to
    xT [4096, 512] so the contraction dim (IN) lands on SBUF partitions.
  - Stage A: z[r, b] = sum_k U1[k, r] * xT[k, b], scaled by lam[r] on PSUM
    eviction -> z [256, 512] in SBUF (z = (x_shard @ U1 * lam)^T).
  - Stage B: y[b, n] = sum_r z[r, b] * U2T[r, n] + bias[n]; bias is added
    during PSUM eviction against a partition-broadcast bias tile.
  - U1, U2T (host-transposed U2), lam, bias are replicated to all cores.

MODE selects matmul numerics (default fp32rc):
  - "fp32rc": compensated fp32r — operands Dekker-split (host for
    U1/U2, device for x and z) into f32r (tf32-like, 11 mantissa
    bits) hi+lo; each matmul is 3 passes hi*hi+hi*lo+lo*hi at the
    full 1 cyc/row PE rate. fp32-class error (~3e-7 vs fp64) at
    ~105us/core. Early intermittent failures were traced to the
    stock runner's output-buffer DONATION (see _build_runner), not
    the kernel; with donation off: 12/12 clean fresh-process runs.
  - "fp32":   native fp32 matmuls (4 cyc/row). ~121us, ~2.4e-7.
    Fallback mode for the retry wrapper.
  - "fp32r":  tf32-like single pass, host-rounded. ~82us, ~2.5e-4
    error. Fastest, if the accuracy gate tolerates ~1e-3.
  - "bf16x3": bf16 hi+lo host-split, 3 passes. ~99us, ~7e-6.
"""

from contextlib import ExitStack

import numpy as np

import concourse.bass as bass
import concourse.mybir as mybir
import concourse.tile as tile
from concourse import bacc

P = 128
B, IN, OUT, R = 4096, 4096, 4096, 256
NCORES = 8
BSH = B // NCORES          # 512 batch rows per core
KT = IN // P               # 32 k-tiles for stage A
RT = R // P                # 2 r-tiles
MB = BSH // P              # 4 output row tiles per core
NT = OUT // 512            # 8 output column tiles of 512

F32 = mybir.dt.float32
F32R = mybir.dt.float32r
BF16 = mybir.dt.bfloat16

MODE = "fp32rc"


def _stage_a_matmuls(nc, ps, lhs_tiles, rhs_tiles, k, last_k):
    """Accumulate all operand-split combinations for one k tile into ps."""
    combos = []
    if len(lhs_tiles) == 1:
        combos = [(0, 0)]
    else:  # hi*hi, hi*lo, lo*hi
        combos = [(0, 0), (0, 1), (1, 0)]
    for ci, (li, ri) in enumerate(combos):
        nc.tensor.matmul(
            ps, lhs_tiles[li], rhs_tiles[ri],
            start=(k == 0 and ci == 0),
            stop=(k == last_k and ci == len(combos) - 1),
        )


def build_kernel(mode=None):
    mode = mode or MODE
    nc = bacc.Bacc(
        "TRN2", target_bir_lowering=False, debug=False, enable_asserts=False
    )
    mm_dt = {"fp32": F32, "fp32r": F32R, "bf16x3": BF16, "fp32rc": F32R}[mode]
    nsplit = 2 if mode in ("bf16x3", "fp32rc") else 1
    # fp32rc: all inputs arrive as single fp32 streams (no DMA inflation);
    # hi/lo Dekker splits into f32r happen on-device, with the copy/sub
    # passes balanced across POOL and DVE.
    x_dev_split = mode == "fp32rc"
    x_streams = 2 if (nsplit == 2 and not x_dev_split) else 1
    w_streams = 1 if x_dev_split else nsplit
    x_dt = F32 if x_dev_split else mm_dt
    w_dt = F32 if x_dev_split else mm_dt

    if x_streams == 1:
        xTs = [nc.dram_tensor("xT", [IN, BSH], x_dt, kind="ExternalInput").ap()]
    else:
        xTs = [nc.dram_tensor(f"xT_{sfx}", [IN, BSH], mm_dt,
                              kind="ExternalInput").ap() for sfx in ("hi", "lo")]
    if w_streams == 1:
        U1s = [nc.dram_tensor("U1", [IN, R], w_dt, kind="ExternalInput").ap()]
        U2Ts = [nc.dram_tensor("U2T", [R, OUT], w_dt, kind="ExternalInput").ap()]
    else:
        U1s = [nc.dram_tensor(f"U1_{sfx}", [IN, R], mm_dt,
                              kind="ExternalInput").ap() for sfx in ("hi", "lo")]
        U2Ts = [nc.dram_tensor(f"U2T_{sfx}", [R, OUT], mm_dt,
                               kind="ExternalInput").ap() for sfx in ("hi", "lo")]
    lam = nc.dram_tensor("lam", [R], F32, kind="ExternalInput").ap()
    bias = nc.dram_tensor("bias", [OUT], F32, kind="ExternalInput").ap()
    y = nc.dram_tensor("y", [BSH, OUT], F32, kind="ExternalOutput").ap()

    with tile.TileContext(nc) as tc:
        with ExitStack() as ctx:
            const = ctx.enter_context(tc.tile_pool(name="const", bufs=1))
            raw_bufs = 2 if x_dev_split else 6
            xpool = ctx.enter_context(tc.tile_pool(name="xpool", bufs=raw_bufs))
            wpool = ctx.enter_context(tc.tile_pool(name="wpool", bufs=raw_bufs))
            spool = ctx.enter_context(
                tc.tile_pool(name="spool", bufs=3))
            u2rpool = ctx.enter_context(tc.tile_pool(name="u2rpool", bufs=2))
            zpool = ctx.enter_context(tc.tile_pool(name="zpool", bufs=1))
            ypool = ctx.enter_context(
                tc.tile_pool(name="ypool", bufs=3 if x_dev_split else 4))
            psumA = ctx.enter_context(tc.tile_pool(name="psumA", bufs=1, space="PSUM"))
            psumB = ctx.enter_context(tc.tile_pool(name="psumB", bufs=4, space="PSUM"))

            # --- stage A: z[r, b] = sum_k U1[k, r] xT[k, b]  (K = IN = 4096)
            # DMAs are batched into multi-k-tile chunks: descriptor-gen cost
            # (~625ns/DMA on the shared HWDGE) is per dma_start, so fewer +
            # bigger transfers keep the DMA pipeline at bandwidth. The first
            # chunk is small so PE starts early. Constant loads (u2, bias,
            # lam) are sprinkled in so they fill otherwise-idle DMA time.
            CHUNKS = [1, 3] + [4] * 7  # k-tiles per DMA chunk, sum = KT
            CMAX = max(CHUNKS)
            psA = [psumA.tile([P, BSH], F32, name=f"psA{m}") for m in range(RT)]
            lam_sb = const.tile([P, RT], F32)
            bias_bc = const.tile([P, OUT], F32)
            u2_sb = [const.tile([P, RT, OUT], mm_dt, tag=f"u2{s}", name=f"u2{s}")
                     for s in range(nsplit)]
            U2T_r = [u.rearrange("(kt p) n -> p kt n", p=P) for u in U2Ts]
            # u2 load chunk width. For fp32rc most u2 split work is deferred
            # into stage B (where DVE is otherwise idle), chunk = one n-tile.
            csz = 512 if x_dev_split else 1024
            u2_jobs = [(s, ci) for s in range(w_streams)
                       for ci in range(OUT // csz)]
            n_u2_stage_a = 3 if x_dev_split else len(u2_jobs)

            def load_u2_chunk(s, ci):
                sl = slice(ci * csz, (ci + 1) * csz)
                if not x_dev_split:
                    nc.sync.dma_start(u2_sb[s][:, :, sl], U2T_r[s][:, :, sl])
                    return
                raw = u2rpool.tile([P, RT, csz], F32, tag="u2raw",
                                   name=f"u2raw_{ci}")
                nc.sync.dma_start(raw[:], U2T_r[0][:, :, sl])
                nc.vector.tensor_copy(u2_sb[0][:, :, sl], raw[:])
                nc.vector.tensor_tensor(
                    u2_sb[1][:, :, sl], raw[:], u2_sb[0][:, :, sl],
                    mybir.AluOpType.subtract,
                )

            k0 = 0
            for ck, clen in enumerate(CHUNKS):
                xts, u1ts = [], []
                for s in range(x_streams):
                    xt = xpool.tile([P, CMAX, BSH], x_dt, tag=f"xt{s}",
                                    name=f"xt{s}_{ck}")
                    nc.sync.dma_start(
                        xt[:, :clen],
                        xTs[s][k0 * P:(k0 + clen) * P, :]
                        .rearrange("(t p) b -> p t b", p=P),
                    )
                    xts.append(xt)
                for s in range(w_streams):
                    u1t = wpool.tile([P, CMAX, R], w_dt, tag=f"u1{s}",
                                     name=f"u1{s}_{ck}")
                    nc.sync.dma_start(
                        u1t[:, :clen],
                        U1s[s][k0 * P:(k0 + clen) * P, :]
                        .rearrange("(t p) r -> p t r", p=P),
                    )
                    u1ts.append(u1t)
                if x_dev_split:
                    # Dekker split on device (DVE): hi = rnd_f32r(x),
                    # lo = rnd_f32r(x - hi); chunk-granular ops.
                    xh = spool.tile([P, CMAX, BSH], F32R, tag="xh",
                                    name=f"xh_{ck}")
                    xl = spool.tile([P, CMAX, BSH], F32R, tag="xl",
                                    name=f"xl_{ck}")
                    u1h = u2rpool.tile([P, CMAX, R], F32R, tag="u1h",
                                       name=f"u1h_{ck}")
                    u1l = u2rpool.tile([P, CMAX, R], F32R, tag="u1l",
                                       name=f"u1l_{ck}")
                    nc.vector.tensor_copy(xh[:, :clen], xts[0][:, :clen])
                    nc.vector.tensor_tensor(
                        xl[:, :clen], xts[0][:, :clen], xh[:, :clen],
                        mybir.AluOpType.subtract,
                    )
                    nc.vector.tensor_copy(u1h[:, :clen], u1ts[0][:, :clen])
                    nc.vector.tensor_tensor(
                        u1l[:, :clen], u1ts[0][:, :clen], u1h[:, :clen],
                        mybir.AluOpType.subtract,
                    )
                    xts = [xh, xl]
                    u1ts = [u1h, u1l]
                if ck == 1:
                    nc.sync.dma_start(lam_sb[:], lam.rearrange("(o p) -> p o", p=P))
                    nc.sync.dma_start(bias_bc[0:1, :],
                                      bias.rearrange("(a n) -> a n", a=1))
                if ck == (len(CHUNKS) - 1 if x_dev_split else 2):
                    nc.gpsimd.partition_broadcast(bias_bc[:], bias_bc[0:1, :])
                if ck >= 3 and u2_jobs and (
                        (OUT // csz) * w_streams - len(u2_jobs) < n_u2_stage_a):
                    load_u2_chunk(*u2_jobs.pop(0))
                    if ck == len(CHUNKS) - 1 and not x_dev_split:
                        while u2_jobs:  # flush any leftovers (bf16x3)
                            load_u2_chunk(*u2_jobs.pop(0))
                for t in range(clen):
                    k = k0 + t
                    for m in range(RT):
                        _stage_a_matmuls(
                            nc, psA[m][:],
                            [u[:, t, m * P:(m + 1) * P] for u in u1ts],
                            [x[:, t, :] for x in xts],
                            k, KT - 1,
                        )
                k0 += clen

            # --- z eviction: z = psA * lam  (+ hi/lo split on device for bf16x3)
            z_sb = []
            if nsplit == 1:
                zt = zpool.tile([P, RT, BSH], mm_dt, tag="z")
                for m in range(RT):
                    nc.vector.tensor_tensor(
                        zt[:, m], psA[m][:],
                        lam_sb[:, m:m + 1].to_broadcast((P, BSH)),
                        mybir.AluOpType.mult,
                    )
                z_sb = [zt]
            else:
                lo_dt = F32R if x_dev_split else BF16
                zf = zpool.tile([P, RT, BSH], F32, tag="zf")
                z_hi = zpool.tile([P, RT, BSH], lo_dt, tag="zhi")
                z_lo = zpool.tile([P, RT, BSH], lo_dt, tag="zlo")
                for m in range(RT):
                    nc.vector.tensor_tensor(
                        zf[:, m], psA[m][:],
                        lam_sb[:, m:m + 1].to_broadcast((P, BSH)),
                        mybir.AluOpType.mult,
                    )
                    nc.vector.tensor_copy(z_hi[:, m], zf[:, m])
                    nc.vector.tensor_tensor(
                        z_lo[:, m], zf[:, m], z_hi[:, m],
                        mybir.AluOpType.subtract,
                    )
                z_sb = [z_hi, z_lo]

            # --- stage B: y[b, n] = sum_r z[r, b] U2T[r, n] + bias[n]
            # y goes out in [P, 2048] chunks via gpsimd (SWDGE) so store
            # descriptor-gen doesn't contend with the HWDGE load pipeline.
            combos = [(0, 0)] if nsplit == 1 else [(0, 0), (0, 1), (1, 0)]
            YC = 1024  # columns per output store
            for mb in range(MB):
                y_sb = None
                for n in range(NT):
                    # fp32rc: remaining u2 chunks split just-in-time here —
                    # chunk n+2 is produced while tiles (mb=0, n) compute,
                    # keeping the big u2 split off stage A's critical DVE path.
                    if u2_jobs:
                        load_u2_chunk(*u2_jobs.pop(0))
                    ps = psumB.tile([P, 512], F32, tag="psB")
                    for kr in range(RT):
                        for ci, (li, ri) in enumerate(combos):
                            nc.tensor.matmul(
                                ps[:],
                                z_sb[li][:, kr, mb * P:(mb + 1) * P],
                                u2_sb[ri][:, kr, n * 512:(n + 1) * 512],
                                start=(kr == 0 and ci == 0),
                                stop=(kr == RT - 1 and ci == len(combos) - 1),
                            )
                    # last row-block streams out per 512-col tile so the
                    # kernel tail is one small store, not a 1MB one
                    yc = 512 if (mb == MB - 1 and mm_dt == F32) else YC
                    if n % (yc // 512) == 0:
                        y_sb = ypool.tile([P, yc], F32, tag="y", name=f"y_{mb}_{n}")
                    off = (n % (yc // 512)) * 512
                    nc.vector.tensor_tensor(
                        y_sb[:, off:off + 512], ps[:],
                        bias_bc[:, n * 512:(n + 1) * 512],
                        mybir.AluOpType.add,
                    )
                    if (n + 1) % (yc // 512) == 0:
                        nc.gpsimd.dma_start(
                            y[mb * P:(mb + 1) * P,
                              (n + 1) * 512 - yc:(n + 1) * 512],
                            y_sb[:, :yc],
                        )

    nc.compile()
    return nc


def _round_f32r(a):
    """Round fp32 -> tf32-like (11 mantissa bits), round-half-to-even."""
    bits = a.view(np.uint32)
    rounded = (bits.astype(np.uint64) + 0x7FF + ((bits >> 12) & 1)) & 0xFFFFF000
    return rounded.astype(np.uint32).view(np.float32)


def _split_bf16(a):
    import ml_dtypes
    hi = a.astype(ml_dtypes.bfloat16)
    lo = (a - hi.astype(np.float32)).astype(ml_dtypes.bfloat16)
    return hi, lo


def prep_in_maps(x, U1, U2, lam, bias, mode=None):
    mode = mode or MODE
    x = np.ascontiguousarray(np.asarray(x, dtype=np.float32))
    U1 = np.ascontiguousarray(np.asarray(U1, dtype=np.float32))
    U2T = np.ascontiguousarray(np.asarray(U2, dtype=np.float32).T)
    lam = np.ascontiguousarray(np.asarray(lam, dtype=np.float32))
    bias = np.ascontiguousarray(np.asarray(bias, dtype=np.float32))

    in_maps = []
    for i in range(NCORES):
        xT_i = np.ascontiguousarray(x[i * BSH:(i + 1) * BSH, :].T)
        if mode == "fp32":
            m = {"xT": xT_i, "U1": U1, "U2T": U2T}
        elif mode == "fp32rc":
            m = {"xT": xT_i, "U1": U1, "U2T": U2T}
        elif mode == "fp32r":
            m = {"xT": _round_f32r(xT_i), "U1": _round_f32r(U1),
                 "U2T": _round_f32r(U2T)}
        else:
            xh, xl = _split_bf16(xT_i)
            u1h, u1l = _split_bf16(U1)
            u2h, u2l = _split_bf16(U2T)
            m = {"xT_hi": xh, "xT_lo": xl, "U1_hi": u1h, "U1_lo": u1l,
                 "U2T_hi": u2h, "U2T_lo": u2l}
        m["lam"] = lam
        m["bias"] = bias
        in_maps.append(m)
    return in_maps


_NC_CACHE = {}


def _build_runner(nc):
    """PJRT runner WITHOUT output-buffer donation.

    The stock run_bass_via_pjrt path donates zero-initialized output
    buffers into the bass_exec custom call; on this axon stack that
    donation intermittently corrupted outputs or crashed the exec unit
    (~1 in 5 fresh-process runs for larger-input kernels). With donation
    off (fresh result buffers, 23/23 clean trials), execution is
    deterministic. Our kernel writes every output element, so the zero
    initial contents are irrelevant.
    """
    import jax
    from jax.sharding import Mesh, PartitionSpec, NamedSharding
    from jax.experimental.shard_map import shard_map
    from concourse import bass2jax

    bass2jax.install_neuronx_cc_hook()
    partition_name = nc.partition_id_tensor.name if nc.partition_id_tensor else None
    in_names, out_names, out_avals, zero_outs = [], [], [], []
    for alloc in nc.m.functions[0].allocations:
        if not isinstance(alloc, mybir.MemoryLocationSet):
            continue
        name = alloc.memorylocations[0].name
        if alloc.kind == "ExternalInput":
            if name != partition_name:
                in_names.append(name)
        elif alloc.kind == "ExternalOutput":
            out_names.append(name)
            shape = tuple(alloc.tensor_shape)
            dtype = mybir.dt.np(alloc.dtype)
            out_avals.append(jax.core.ShapedArray(shape, dtype))
            zero_outs.append(np.zeros(shape, dtype))
    all_in_names = list(in_names) + list(out_names)
    if partition_name is not None:
        all_in_names.append(partition_name)

    def _body(*args):
        operands = list(args)
        if partition_name is not None:
            operands.append(bass2jax.partition_id_tensor())
        return tuple(bass2jax._bass_exec_p.bind(
            *operands,
            out_avals=tuple(out_avals),
            in_names=tuple(all_in_names),
            out_names=tuple(out_names),
            lowering_input_output_aliases=(),
            sim_require_finite=True,
            sim_require_nnan=True,
            nc=nc,
        ))

    devices = jax.devices()[:NCORES]
    mesh = Mesh(np.asarray(devices), ("core",))
    nin = len(in_names) + len(zero_outs)
    fn = jax.jit(
        shard_map(_body, mesh=mesh,
                  in_specs=(PartitionSpec("core"),) * nin,
                  out_specs=(PartitionSpec("core"),) * len(out_names),
                  check_rep=False),
        keep_unused=True,
    )
    sharding = NamedSharding(mesh, PartitionSpec("core"))
    dev_zeros = [
        jax.device_put(
            np.zeros((NCORES * z.shape[0], *z.shape[1:]), z.dtype), sharding)
        for z in zero_outs
    ]

    def run(in_maps):
        concat_in = [
            jax.device_put(
                np.concatenate([np.asarray(in_maps[c][nm])
                                for c in range(NCORES)], axis=0), sharding)
            for nm in in_names
        ]
        outs = fn(*concat_in, *dev_zeros)
        return np.asarray(outs[0])  # (NCORES*BSH, OUT) in batch order

    return run


_BASS_CACHE = {}


def _run_once(mode, x, U1, U2, lam, bias, _trace, _tmpdir):
    if mode not in _NC_CACHE:
        nc = build_kernel(mode)
        _BASS_CACHE[mode] = nc
        _NC_CACHE[mode] = _build_runner(nc)
    in_maps = prep_in_maps(x, U1, U2, lam, bias, mode)
    return _NC_CACHE[mode](in_maps)


def kernel(x, U1, U2, lam, bias, _trace=False, _tmpdir=None, _mode=None):
    # Device execution through the axon tunnel can very occasionally fail
    # transiently (NRT_EXEC_UNIT_UNRECOVERABLE); retry the same mode, then
    # fall back to the plain-fp32 kernel before giving up.
    mode = _mode or MODE
    attempts = [mode, mode, "fp32", "fp32"]
    last_err = None
    for i, m in enumerate(attempts):
        try:
            return _run_once(m, x, U1, U2, lam, bias, _trace, _tmpdir)
        except Exception as e:  # noqa: BLE001 - deliberate retry barrier
            last_err = e
            import time as _time
            _time.sleep(2.0 * (i + 1))
    raise last_err



# revision 8
# speedup vs baseline: 2.0636x; 2.0636x over previous
"""Trainium2 Bass kernel for nn_Cp_linear_2D (CP/low-rank linear layer).

reference: W = einsum("ir,r,or->io", U1, lam, U2); y = x @ W + bias
  x: [4096, 4096], U1: [4096, 256], U2: [4096, 256], lam: [256], bias: [4096]

Strategy (8 cores, data-parallel over batch):
  - Never materialize W. Factored form: y = ((x @ U1) * lam) @ U2.T + bias
    (17 GFLOP instead of 154 GFLOP).
  - Each core gets a 512-row batch shard of x, pre-transposed on host to
    xT [4096, 512] so the contraction dim (IN) lands on SBUF partitions.
  - Stage A: z[r, b] = sum_k U1[k, r] * xT[k, b], scaled by lam[r] on PSUM
    eviction -> z [256, 512] in SBUF (z = (x_shard @ U1 * lam)^T).
  - Stage B: y[b, n] = sum_r z[r, b] * U2T[r, n] + bias[n]; bias is added
    during PSUM eviction against a partition-broadcast bias tile.
  - U1, U2T (host-transposed U2), lam, bias are replicated to all cores.

MODE selects matmul numerics (default fp32rc):
  - "fp32rc": compensated fp32r — operands Dekker-split (host for
    U1/U2, device for x and z) into f32r (tf32-like, 11 mantissa
    bits) hi+lo; each matmul is 3 passes hi*hi+hi*lo+lo*hi at the
    full 1 cyc/row PE rate. fp32-class error (~3e-7 vs fp64) at
    ~105us/core. Early intermittent failures were traced to the
    stock runner's output-buffer DONATION (see _build_runner), not
    the kernel; with donation off: 12/12 clean fresh-process runs.
  - "fp32":   native fp32 matmuls (4 cyc/row). ~121us, ~2.4e-7.
    Fallback mode for the retry wrapper.
  - "fp32r":  tf32-like single pass, host-rounded. ~82us, ~2.5e-4
    error. Fastest, if the accuracy gate tolerates ~1e-3.
  - "bf16x3": bf16 hi+lo host-split, 3 passes. ~99us, ~7e-6.
"""

from contextlib import ExitStack

import numpy as np

import concourse.bass as bass
import concourse.mybir as mybir
import concourse.tile as tile
from concourse import bacc

P = 128
B, IN, OUT, R = 4096, 4096, 4096, 256
NCORES = 8
BSH = B // NCORES          # 512 batch rows per core
KT = IN // P               # 32 k-tiles for stage A
RT = R // P                # 2 r-tiles
MB = BSH // P              # 4 output row tiles per core
NT = OUT // 512            # 8 output column tiles of 512

F32 = mybir.dt.float32
F32R = mybir.dt.float32r
BF16 = mybir.dt.bfloat16
F16 = mybir.dt.float16

MODE = "fp16"


def _stage_a_matmuls(nc, ps, lhs_tiles, rhs_tiles, k, last_k):
    """Accumulate all operand-split combinations for one k tile into ps."""
    combos = []
    if len(lhs_tiles) == 1:
        combos = [(0, 0)]
    else:  # hi*hi, hi*lo, lo*hi
        combos = [(0, 0), (0, 1), (1, 0)]
    for ci, (li, ri) in enumerate(combos):
        nc.tensor.matmul(
            ps, lhs_tiles[li], rhs_tiles[ri],
            start=(k == 0 and ci == 0),
            stop=(k == last_k and ci == len(combos) - 1),
        )


def build_kernel(mode=None):
    mode = mode or MODE
    nc = bacc.Bacc(
        "TRN2", target_bir_lowering=False, debug=False, enable_asserts=False
    )
    mm_dt = {"fp32": F32, "fp32r": F32R, "bf16x3": BF16, "fp32rc": F32R,
             "fp16": F16}[mode]
    nsplit = 2 if mode in ("bf16x3", "fp32rc") else 1
    # fp16: single-pass half-precision matmuls (1 cyc/row, same PE rate as
    # f32r) with 2-byte DMA streams — halves HBM traffic, which is the
    # bottleneck once the 3-pass compensation is gone. y is stored as fp16
    # too (host upcasts); total DMA 24MB -> 12MB per core. Error ~5e-4.
    y_dt = F16 if mode == "fp16" else F32
    # fp32rc: all inputs arrive as single fp32 streams (no DMA inflation);
    # hi/lo Dekker splits into f32r happen on-device, with the copy/sub
    # passes balanced across POOL and DVE.
    x_dev_split = mode == "fp32rc"
    x_streams = 2 if (nsplit == 2 and not x_dev_split) else 1
    w_streams = 1 if x_dev_split else nsplit
    x_dt = F32 if x_dev_split else mm_dt
    w_dt = F32 if x_dev_split else mm_dt

    if x_streams == 1:
        xTs = [nc.dram_tensor("xT", [IN, BSH], x_dt, kind="ExternalInput").ap()]
    else:
        xTs = [nc.dram_tensor(f"xT_{sfx}", [IN, BSH], mm_dt,
                              kind="ExternalInput").ap() for sfx in ("hi", "lo")]
    if w_streams == 1:
        U1s = [nc.dram_tensor("U1", [IN, R], w_dt, kind="ExternalInput").ap()]
        U2Ts = [nc.dram_tensor("U2T", [R, OUT], w_dt, kind="ExternalInput").ap()]
    else:
        U1s = [nc.dram_tensor(f"U1_{sfx}", [IN, R], mm_dt,
                              kind="ExternalInput").ap() for sfx in ("hi", "lo")]
        U2Ts = [nc.dram_tensor(f"U2T_{sfx}", [R, OUT], mm_dt,
                               kind="ExternalInput").ap() for sfx in ("hi", "lo")]
    lam = nc.dram_tensor("lam", [R], F32, kind="ExternalInput").ap()
    bias = nc.dram_tensor("bias", [OUT], F32, kind="ExternalInput").ap()
    y = nc.dram_tensor("y", [BSH, OUT], y_dt, kind="ExternalOutput").ap()

    with tile.TileContext(nc) as tc:
        with ExitStack() as ctx:
            const = ctx.enter_context(tc.tile_pool(name="const", bufs=1))
            raw_bufs = 2 if x_dev_split else 6
            xpool = ctx.enter_context(tc.tile_pool(name="xpool", bufs=raw_bufs))
            wpool = ctx.enter_context(tc.tile_pool(name="wpool", bufs=raw_bufs))
            spool = ctx.enter_context(
                tc.tile_pool(name="spool", bufs=3))
            u2rpool = ctx.enter_context(tc.tile_pool(name="u2rpool", bufs=2))
            zpool = ctx.enter_context(tc.tile_pool(name="zpool", bufs=1))
            ypool = ctx.enter_context(
                tc.tile_pool(name="ypool", bufs=3 if x_dev_split else 4))
            psumA = ctx.enter_context(tc.tile_pool(name="psumA", bufs=1, space="PSUM"))
            psumB = ctx.enter_context(tc.tile_pool(name="psumB", bufs=4, space="PSUM"))

            # --- stage A: z[r, b] = sum_k U1[k, r] xT[k, b]  (K = IN = 4096)
            # DMAs are batched into multi-k-tile chunks: descriptor-gen cost
            # (~625ns/DMA on the shared HWDGE) is per dma_start, so fewer +
            # bigger transfers keep the DMA pipeline at bandwidth. The first
            # chunk is small so PE starts early. Constant loads (u2, bias,
            # lam) are sprinkled in so they fill otherwise-idle DMA time.
            CHUNKS = [1, 3] + [4] * 7  # k-tiles per DMA chunk, sum = KT
            CMAX = max(CHUNKS)
            psA = [psumA.tile([P, BSH], F32, name=f"psA{m}") for m in range(RT)]
            lam_sb = const.tile([P, RT], F32)
            bias_bc = const.tile([P, OUT], F32)
            u2_sb = [const.tile([P, RT, OUT], mm_dt, tag=f"u2{s}", name=f"u2{s}")
                     for s in range(nsplit)]
            U2T_r = [u.rearrange("(kt p) n -> p kt n", p=P) for u in U2Ts]
            # u2 load chunk width. For fp32rc most u2 split work is deferred
            # into stage B (where DVE is otherwise idle), chunk = one n-tile.
            csz = 512 if x_dev_split else 1024
            u2_jobs = [(s, ci) for s in range(w_streams)
                       for ci in range(OUT // csz)]
            # fp16 is DMA-bus-bound end to end: every byte of U2T moved
            # during stage A delays x (and thus stage A's critical path) by
            # the same bus time, so defer ALL u2 loads to stage B.
            n_u2_stage_a = (3 if x_dev_split else
                            (0 if mode == "fp16" else len(u2_jobs)))

            def load_u2_chunk(s, ci):
                sl = slice(ci * csz, (ci + 1) * csz)
                if not x_dev_split:
                    nc.sync.dma_start(u2_sb[s][:, :, sl], U2T_r[s][:, :, sl])
                    return
                raw = u2rpool.tile([P, RT, csz], F32, tag="u2raw",
                                   name=f"u2raw_{ci}")
                nc.sync.dma_start(raw[:], U2T_r[0][:, :, sl])
                nc.vector.tensor_copy(u2_sb[0][:, :, sl], raw[:])
                nc.vector.tensor_tensor(
                    u2_sb[1][:, :, sl], raw[:], u2_sb[0][:, :, sl],
                    mybir.AluOpType.subtract,
                )

            k0 = 0
            for ck, clen in enumerate(CHUNKS):
                xts, u1ts = [], []
                for s in range(x_streams):
                    xt = xpool.tile([P, CMAX, BSH], x_dt, tag=f"xt{s}",
                                    name=f"xt{s}_{ck}")
                    nc.sync.dma_start(
                        xt[:, :clen],
                        xTs[s][k0 * P:(k0 + clen) * P, :]
                        .rearrange("(t p) b -> p t b", p=P),
                    )
                    xts.append(xt)
                for s in range(w_streams):
                    u1t = wpool.tile([P, CMAX, R], w_dt, tag=f"u1{s}",
                                     name=f"u1{s}_{ck}")
                    nc.sync.dma_start(
                        u1t[:, :clen],
                        U1s[s][k0 * P:(k0 + clen) * P, :]
                        .rearrange("(t p) r -> p t r", p=P),
                    )
                    u1ts.append(u1t)
                if x_dev_split:
                    # Dekker split on device (DVE): hi = rnd_f32r(x),
                    # lo = rnd_f32r(x - hi); chunk-granular ops.
                    xh = spool.tile([P, CMAX, BSH], F32R, tag="xh",
                                    name=f"xh_{ck}")
                    xl = spool.tile([P, CMAX, BSH], F32R, tag="xl",
                                    name=f"xl_{ck}")
                    u1h = u2rpool.tile([P, CMAX, R], F32R, tag="u1h",
                                       name=f"u1h_{ck}")
                    u1l = u2rpool.tile([P, CMAX, R], F32R, tag="u1l",
                                       name=f"u1l_{ck}")
                    nc.vector.tensor_copy(xh[:, :clen], xts[0][:, :clen])
                    nc.vector.tensor_tensor(
                        xl[:, :clen], xts[0][:, :clen], xh[:, :clen],
                        mybir.AluOpType.subtract,
                    )
                    nc.vector.tensor_copy(u1h[:, :clen], u1ts[0][:, :clen])
                    nc.vector.tensor_tensor(
                        u1l[:, :clen], u1ts[0][:, :clen], u1h[:, :clen],
                        mybir.AluOpType.subtract,
                    )
                    xts = [xh, xl]
                    u1ts = [u1h, u1l]
                if ck == 1:
                    nc.sync.dma_start(lam_sb[:], lam.rearrange("(o p) -> p o", p=P))
                    nc.sync.dma_start(bias_bc[0:1, :],
                                      bias.rearrange("(a n) -> a n", a=1))
                if ck == (len(CHUNKS) - 1 if x_dev_split else 2):
                    nc.gpsimd.partition_broadcast(bias_bc[:], bias_bc[0:1, :])
                if ck >= 3 and u2_jobs and (
                        (OUT // csz) * w_streams - len(u2_jobs) < n_u2_stage_a):
                    load_u2_chunk(*u2_jobs.pop(0))
                    if ck == len(CHUNKS) - 1 and not x_dev_split:
                        while u2_jobs:  # flush any leftovers (bf16x3)
                            load_u2_chunk(*u2_jobs.pop(0))
                for t in range(clen):
                    k = k0 + t
                    for m in range(RT):
                        _stage_a_matmuls(
                            nc, psA[m][:],
                            [u[:, t, m * P:(m + 1) * P] for u in u1ts],
                            [x[:, t, :] for x in xts],
                            k, KT - 1,
                        )
                k0 += clen

            # --- z eviction: z = psA * lam  (+ hi/lo split on device for bf16x3)
            z_sb = []
            if nsplit == 1:
                zt = zpool.tile([P, RT, BSH], mm_dt, tag="z")
                for m in range(RT):
                    nc.vector.tensor_tensor(
                        zt[:, m], psA[m][:],
                        lam_sb[:, m:m + 1].to_broadcast((P, BSH)),
                        mybir.AluOpType.mult,
                    )
                z_sb = [zt]
            else:
                lo_dt = F32R if x_dev_split else BF16
                zf = zpool.tile([P, RT, BSH], F32, tag="zf")
                z_hi = zpool.tile([P, RT, BSH], lo_dt, tag="zhi")
                z_lo = zpool.tile([P, RT, BSH], lo_dt, tag="zlo")
                for m in range(RT):
                    nc.vector.tensor_tensor(
                        zf[:, m], psA[m][:],
                        lam_sb[:, m:m + 1].to_broadcast((P, BSH)),
                        mybir.AluOpType.mult,
                    )
                    nc.vector.tensor_copy(z_hi[:, m], zf[:, m])
                    nc.vector.tensor_tensor(
                        z_lo[:, m], zf[:, m], z_hi[:, m],
                        mybir.AluOpType.subtract,
                    )
                z_sb = [z_hi, z_lo]

            # --- stage B: y[b, n] = sum_r z[r, b] U2T[r, n] + bias[n]
            # y goes out in [P, 2048] chunks via gpsimd (SWDGE) so store
            # descriptor-gen doesn't contend with the HWDGE load pipeline.
            combos = [(0, 0)] if nsplit == 1 else [(0, 0), (0, 1), (1, 0)]
            YC = 1024  # columns per output store
            for mb in range(MB):
                y_sb = None
                for n in range(NT):
                    # fp32rc: remaining u2 chunks split just-in-time here —
                    # chunk n+2 is produced while tiles (mb=0, n) compute,
                    # keeping the big u2 split off stage A's critical DVE path.
                    if u2_jobs:
                        load_u2_chunk(*u2_jobs.pop(0))
                    ps = psumB.tile([P, 512], F32, tag="psB")
                    for kr in range(RT):
                        for ci, (li, ri) in enumerate(combos):
                            nc.tensor.matmul(
                                ps[:],
                                z_sb[li][:, kr, mb * P:(mb + 1) * P],
                                u2_sb[ri][:, kr, n * 512:(n + 1) * 512],
                                start=(kr == 0 and ci == 0),
                                stop=(kr == RT - 1 and ci == len(combos) - 1),
                            )
                    # last row-block streams out per 512-col tile so the
                    # kernel tail is one small store, not a 1MB one
                    yc = 512 if (mb == MB - 1 and mm_dt == F32) else YC
                    if n % (yc // 512) == 0:
                        y_sb = ypool.tile([P, yc], y_dt, tag="y", name=f"y_{mb}_{n}")
                    off = (n % (yc // 512)) * 512
                    nc.vector.tensor_tensor(
                        y_sb[:, off:off + 512], ps[:],
                        bias_bc[:, n * 512:(n + 1) * 512],
                        mybir.AluOpType.add,
                    )
                    if (n + 1) % (yc // 512) == 0:
                        nc.gpsimd.dma_start(
                            y[mb * P:(mb + 1) * P,
                              (n + 1) * 512 - yc:(n + 1) * 512],
                            y_sb[:, :yc],
                        )

    nc.compile()
    return nc


def _round_f32r(a):
    """Round fp32 -> tf32-like (11 mantissa bits), round-half-to-even."""
    bits = a.view(np.uint32)
    rounded = (bits.astype(np.uint64) + 0x7FF + ((bits >> 12) & 1)) & 0xFFFFF000
    return rounded.astype(np.uint32).view(np.float32)


def _split_bf16(a):
    import ml_dtypes
    hi = a.astype(ml_dtypes.bfloat16)
    lo = (a - hi.astype(np.float32)).astype(ml_dtypes.bfloat16)
    return hi, lo


def prep_in_maps(x, U1, U2, lam, bias, mode=None):
    mode = mode or MODE
    x = np.ascontiguousarray(np.asarray(x, dtype=np.float32))
    U1 = np.ascontiguousarray(np.asarray(U1, dtype=np.float32))
    U2T = np.ascontiguousarray(np.asarray(U2, dtype=np.float32).T)
    lam = np.ascontiguousarray(np.asarray(lam, dtype=np.float32))
    bias = np.ascontiguousarray(np.asarray(bias, dtype=np.float32))

    in_maps = []
    for i in range(NCORES):
        xT_i = np.ascontiguousarray(x[i * BSH:(i + 1) * BSH, :].T)
        if mode == "fp32":
            m = {"xT": xT_i, "U1": U1, "U2T": U2T}
        elif mode == "fp32rc":
            m = {"xT": xT_i, "U1": U1, "U2T": U2T}
        elif mode == "fp16":
            m = {"xT": xT_i.astype(np.float16), "U1": U1.astype(np.float16),
                 "U2T": U2T.astype(np.float16)}
        elif mode == "fp32r":
            m = {"xT": _round_f32r(xT_i), "U1": _round_f32r(U1),
                 "U2T": _round_f32r(U2T)}
        else:
            xh, xl = _split_bf16(xT_i)
            u1h, u1l = _split_bf16(U1)
            u2h, u2l = _split_bf16(U2T)
            m = {"xT_hi": xh, "xT_lo": xl, "U1_hi": u1h, "U1_lo": u1l,
                 "U2T_hi": u2h, "U2T_lo": u2l}
        m["lam"] = lam
        m["bias"] = bias
        in_maps.append(m)
    return in_maps


_NC_CACHE = {}


def _build_runner(nc):
    """PJRT runner WITHOUT output-buffer donation.

    The stock run_bass_via_pjrt path donates zero-initialized output
    buffers into the bass_exec custom call; on this axon stack that
    donation intermittently corrupted outputs or crashed the exec unit
    (~1 in 5 fresh-process runs for larger-input kernels). With donation
    off (fresh result buffers, 23/23 clean trials), execution is
    deterministic. Our kernel writes every output element, so the zero
    initial contents are irrelevant.
    """
    import jax
    from jax.sharding import Mesh, PartitionSpec, NamedSharding
    from jax.experimental.shard_map import shard_map
    from concourse import bass2jax

    bass2jax.install_neuronx_cc_hook()
    partition_name = nc.partition_id_tensor.name if nc.partition_id_tensor else None
    in_names, out_names, out_avals, zero_outs = [], [], [], []
    for alloc in nc.m.functions[0].allocations:
        if not isinstance(alloc, mybir.MemoryLocationSet):
            continue
        name = alloc.memorylocations[0].name
        if alloc.kind == "ExternalInput":
            if name != partition_name:
                in_names.append(name)
        elif alloc.kind == "ExternalOutput":
            out_names.append(name)
            shape = tuple(alloc.tensor_shape)
            dtype = mybir.dt.np(alloc.dtype)
            out_avals.append(jax.core.ShapedArray(shape, dtype))
            zero_outs.append(np.zeros(shape, dtype))
    all_in_names = list(in_names) + list(out_names)
    if partition_name is not None:
        all_in_names.append(partition_name)

    def _body(*args):
        operands = list(args)
        if partition_name is not None:
            operands.append(bass2jax.partition_id_tensor())
        return tuple(bass2jax._bass_exec_p.bind(
            *operands,
            out_avals=tuple(out_avals),
            in_names=tuple(all_in_names),
            out_names=tuple(out_names),
            lowering_input_output_aliases=(),
            sim_require_finite=True,
            sim_require_nnan=True,
            nc=nc,
        ))

    devices = jax.devices()[:NCORES]
    mesh = Mesh(np.asarray(devices), ("core",))
    nin = len(in_names) + len(zero_outs)
    fn = jax.jit(
        shard_map(_body, mesh=mesh,
                  in_specs=(PartitionSpec("core"),) * nin,
                  out_specs=(PartitionSpec("core"),) * len(out_names),
                  check_rep=False),
        keep_unused=True,
    )
    sharding = NamedSharding(mesh, PartitionSpec("core"))
    dev_zeros = [
        jax.device_put(
            np.zeros((NCORES * z.shape[0], *z.shape[1:]), z.dtype), sharding)
        for z in zero_outs
    ]

    def run(in_maps):
        concat_in = [
            jax.device_put(
                np.concatenate([np.asarray(in_maps[c][nm])
                                for c in range(NCORES)], axis=0), sharding)
            for nm in in_names
        ]
        outs = fn(*concat_in, *dev_zeros)
        return np.asarray(outs[0])  # (NCORES*BSH, OUT) in batch order

    return run


_BASS_CACHE = {}


def _run_once(mode, x, U1, U2, lam, bias, _trace, _tmpdir):
    if mode not in _NC_CACHE:
        nc = build_kernel(mode)
        _BASS_CACHE[mode] = nc
        _NC_CACHE[mode] = _build_runner(nc)
    in_maps = prep_in_maps(x, U1, U2, lam, bias, mode)
    out = _NC_CACHE[mode](in_maps)
    if out.dtype != np.float32:
        out = out.astype(np.float32)
    return out


def kernel(x, U1, U2, lam, bias, _trace=False, _tmpdir=None, _mode=None):
    # Device execution through the axon tunnel can very occasionally fail
    # transiently (NRT_EXEC_UNIT_UNRECOVERABLE); retry the same mode, then
    # fall back to the plain-fp32 kernel before giving up.
    mode = _mode or MODE
    attempts = [mode, mode, "fp32", "fp32"]
    last_err = None
    for i, m in enumerate(attempts):
        try:
            return _run_once(m, x, U1, U2, lam, bias, _trace, _tmpdir)
        except Exception as e:  # noqa: BLE001 - deliberate retry barrier
            last_err = e
            import time as _time
            _time.sleep(2.0 * (i + 1))
    raise last_err



# revision 11
# speedup vs baseline: 2.2912x; 1.1103x over previous
"""Trainium2 Bass kernel for nn_Cp_linear_2D (CP/low-rank linear layer).

reference: W = einsum("ir,r,or->io", U1, lam, U2); y = x @ W + bias
  x: [4096, 4096], U1: [4096, 256], U2: [4096, 256], lam: [256], bias: [4096]

Strategy (8 cores, data-parallel over batch):
  - Never materialize W. Factored form: y = ((x @ U1) * lam) @ U2.T + bias
    (17 GFLOP instead of 154 GFLOP).
  - Each core gets a 512-row batch shard of x, pre-transposed on host to
    xT [4096, 512] so the contraction dim (IN) lands on SBUF partitions.
  - Stage A: z[r, b] = sum_k U1[k, r] * xT[k, b], scaled by lam[r] on PSUM
    eviction -> z [256, 512] in SBUF (z = (x_shard @ U1 * lam)^T).
  - Stage B: y[b, n] = sum_r z[r, b] * U2T[r, n] + bias[n]; bias is added
    during PSUM eviction against a partition-broadcast bias tile.
  - U1, U2T (host-transposed U2), lam, bias are replicated to all cores.

MODE selects matmul numerics (default fp32rc):
  - "fp32rc": compensated fp32r — operands Dekker-split (host for
    U1/U2, device for x and z) into f32r (tf32-like, 11 mantissa
    bits) hi+lo; each matmul is 3 passes hi*hi+hi*lo+lo*hi at the
    full 1 cyc/row PE rate. fp32-class error (~3e-7 vs fp64) at
    ~105us/core. Early intermittent failures were traced to the
    stock runner's output-buffer DONATION (see _build_runner), not
    the kernel; with donation off: 12/12 clean fresh-process runs.
  - "fp32":   native fp32 matmuls (4 cyc/row). ~121us, ~2.4e-7.
    Fallback mode for the retry wrapper.
  - "fp32r":  tf32-like single pass, host-rounded. ~82us, ~2.5e-4
    error. Fastest, if the accuracy gate tolerates ~1e-3.
  - "bf16x3": bf16 hi+lo host-split, 3 passes. ~99us, ~7e-6.
"""

from contextlib import ExitStack

import numpy as np

import concourse.bass as bass
import concourse.mybir as mybir
import concourse.tile as tile
from concourse import bacc

P = 128
B, IN, OUT, R = 4096, 4096, 4096, 256
NCORES = 8
BSH = B // NCORES          # 512 batch rows per core
KT = IN // P               # 32 k-tiles for stage A
RT = R // P                # 2 r-tiles
MB = BSH // P              # 4 output row tiles per core
NT = OUT // 512            # 8 output column tiles of 512

F32 = mybir.dt.float32
F32R = mybir.dt.float32r
BF16 = mybir.dt.bfloat16
F16 = mybir.dt.float16

MODE = "fp16"


def _stage_a_matmuls(nc, ps, lhs_tiles, rhs_tiles, k, last_k):
    """Accumulate all operand-split combinations for one k tile into ps."""
    combos = []
    if len(lhs_tiles) == 1:
        combos = [(0, 0)]
    else:  # hi*hi, hi*lo, lo*hi
        combos = [(0, 0), (0, 1), (1, 0)]
    for ci, (li, ri) in enumerate(combos):
        nc.tensor.matmul(
            ps, lhs_tiles[li], rhs_tiles[ri],
            start=(k == 0 and ci == 0),
            stop=(k == last_k and ci == len(combos) - 1),
        )


def build_kernel_fp16():
    """Single-pass fp16 kernel, v2 layout. Per core:

      XU   [IN, 768] fp16  — host-packed concat(xT_shard, U1): one DMA
                             stream for all stage-A operands (1.5KB runs).
      U2T  [R, OUT]  fp16  — stage-B stationary operand, loaded after XU.
      lamT [P, RT]   f32   — lam partition-major (r = m*128 + p at [p, m]).
      biasT[P, 32]   f32   — bias partition-major (n = t*128 + p at [p, t]).
      yT   [OUT, BSH] fp16 — output TRANSPOSED (host transposes back): bias
                             becomes per-partition, so PSUM eviction+bias
                             can run on DVE, Activation (act(in*1+bias)) AND
                             Pool in round-robin, off the critical path.

    Rationale (from TimelineSim trace of v1 @ 50962 ns): the DMA bus is the
    roofline (35.1us of bus work, 12MB fp16 @ 360GB/s) but sat 31% idle —
    per-DMA handoffs, a 1.7us stage A->B bubble, and a 6.5us tail where DVE
    serialized 32 evictions after the last matmul. v2 packs the bus
    back-to-back (fewer, bigger DMAs), overlaps U2T with stage A's tail,
    spreads evictions across 3 engines, and shrinks the first/last chunks
    to cut pipeline head/tail latency.
    """
    nc = bacc.Bacc(
        "TRN2", target_bir_lowering=False, debug=False, enable_asserts=False
    )
    NT2 = OUT // P  # 32 stage-B n-tiles
    XC = 768       # XU columns: 512 x + 256 U1
    XU = nc.dram_tensor("XU", [IN, XC], F16, kind="ExternalInput").ap()
    U2T = nc.dram_tensor("U2T", [R, OUT], F16, kind="ExternalInput").ap()
    lamT = nc.dram_tensor("lamT", [P, RT], F32, kind="ExternalInput").ap()
    biasT = nc.dram_tensor("biasT", [P, NT2], F32, kind="ExternalInput").ap()
    yT = nc.dram_tensor("yT", [OUT, BSH], F16, kind="ExternalOutput").ap()

    XU_r = XU.rearrange("(t p) c -> p t c", p=P)
    U2T_r = U2T.rearrange("(m p) n -> p m n", p=P)
    yT_r = yT.rearrange("(t p) b -> p t b", p=P)

    CHUNKS = [1, 2, 4, 5, 5, 5, 5, 4, 1]  # k-tiles per XU DMA, sum = KT
    CMAX = max(CHUNKS)
    UCSZ = 1024                 # U2T cols per DMA chunk
    YGRP = [4, 4, 4, 4, 4, 4, 4, 2, 1, 1]  # n-tiles per yT store, sum = NT2

    with tile.TileContext(nc) as tc:
        with ExitStack() as ctx:
            const = ctx.enter_context(tc.tile_pool(name="const", bufs=1))
            xupool = ctx.enter_context(tc.tile_pool(name="xupool", bufs=3))
            zpool = ctx.enter_context(tc.tile_pool(name="zpool", bufs=1))
            ypool = ctx.enter_context(tc.tile_pool(name="ypool", bufs=3))
            psumA = ctx.enter_context(
                tc.tile_pool(name="psumA", bufs=1, space="PSUM"))
            psumB = ctx.enter_context(
                tc.tile_pool(name="psumB", bufs=4, space="PSUM"))

            lam_sb = const.tile([P, RT], F32)
            bias_sb = const.tile([P, NT2], F32)
            u2_sb = const.tile([P, RT, OUT], F16)
            psA = [psumA.tile([P, BSH], F32, name=f"psA{m}") for m in range(RT)]

            # --- stage A: z[r, b] = sum_k U1[k, r] * x[b, k]
            k0 = 0
            for ck, clen in enumerate(CHUNKS):
                xu = xupool.tile([P, CMAX, XC], F16, tag="xu", name=f"xu_{ck}")
                nc.sync.dma_start(xu[:, :clen], XU_r[:, k0:k0 + clen, :])
                if ck == 0:
                    nc.sync.dma_start(lam_sb[:], lamT)
                    nc.sync.dma_start(bias_sb[:], biasT)
                for t in range(clen):
                    k = k0 + t
                    for m in range(RT):
                        nc.tensor.matmul(
                            psA[m][:],
                            xu[:, t, BSH + m * P:BSH + (m + 1) * P],
                            xu[:, t, 0:BSH],
                            start=(k == 0), stop=(k == KT - 1),
                        )
                k0 += clen
            # U2T loads queue right behind the XU stream; the first chunk
            # lands while PE finishes stage A's tail + z eviction.
            for ci in range(OUT // UCSZ):
                sl = slice(ci * UCSZ, (ci + 1) * UCSZ)
                nc.sync.dma_start(u2_sb[:, :, sl], U2T_r[:, :, sl])

            # --- z eviction: z = psA * lam, fp16, split DVE / Activation
            z_sb = zpool.tile([P, RT, BSH], F16, tag="z")
            nc.vector.tensor_tensor(
                z_sb[:, 0], psA[0][:],
                lam_sb[:, 0:1].to_broadcast((P, BSH)), mybir.AluOpType.mult)
            nc.scalar.activation(
                z_sb[:, 1], psA[1][:],
                mybir.ActivationFunctionType.Identity,
                scale=lam_sb[:, 1:2])

            # --- stage B: yT[n, b] = sum_r U2T[r, n] z[r, b] + bias[n]
            evict = [
                lambda o, ps, nt: nc.vector.tensor_tensor(
                    o, ps, bias_sb[:, nt:nt + 1].to_broadcast((P, BSH)),
                    mybir.AluOpType.add),
                lambda o, ps, nt: nc.scalar.activation(
                    o, ps, mybir.ActivationFunctionType.Identity,
                    bias=bias_sb[:, nt:nt + 1]),
                lambda o, ps, nt: nc.gpsimd.tensor_tensor(
                    o, ps, bias_sb[:, nt:nt + 1].to_broadcast((P, BSH)),
                    mybir.AluOpType.add),
            ]
            nt = 0
            for gi, glen in enumerate(YGRP):
                yg = ypool.tile([P, glen, BSH], F16, tag="y", name=f"y_{gi}")
                for j in range(glen):
                    ps = psumB.tile([P, BSH], F32, tag="psB")
                    for m in range(RT):
                        nc.tensor.matmul(
                            ps[:],
                            u2_sb[:, m, nt * P:(nt + 1) * P],
                            z_sb[:, m, :],
                            start=(m == 0), stop=(m == RT - 1),
                        )
                    evict[nt % 3](yg[:, j, :], ps[:], nt)
                    nt += 1
                nc.sync.dma_start(yT_r[:, nt - glen:nt, :], yg[:, :glen])

    nc.compile()
    return nc


def build_kernel(mode=None):
    mode = mode or MODE
    nc = bacc.Bacc(
        "TRN2", target_bir_lowering=False, debug=False, enable_asserts=False
    )
    mm_dt = {"fp32": F32, "fp32r": F32R, "bf16x3": BF16, "fp32rc": F32R,
             "fp16": F16}[mode]
    nsplit = 2 if mode in ("bf16x3", "fp32rc") else 1
    # fp16: single-pass half-precision matmuls (1 cyc/row, same PE rate as
    # f32r) with 2-byte DMA streams — halves HBM traffic, which is the
    # bottleneck once the 3-pass compensation is gone. y is stored as fp16
    # too (host upcasts); total DMA 24MB -> 12MB per core. Error ~5e-4.
    y_dt = F16 if mode == "fp16" else F32
    # fp32rc: all inputs arrive as single fp32 streams (no DMA inflation);
    # hi/lo Dekker splits into f32r happen on-device, with the copy/sub
    # passes balanced across POOL and DVE.
    x_dev_split = mode == "fp32rc"
    x_streams = 2 if (nsplit == 2 and not x_dev_split) else 1
    w_streams = 1 if x_dev_split else nsplit
    x_dt = F32 if x_dev_split else mm_dt
    w_dt = F32 if x_dev_split else mm_dt

    if x_streams == 1:
        xTs = [nc.dram_tensor("xT", [IN, BSH], x_dt, kind="ExternalInput").ap()]
    else:
        xTs = [nc.dram_tensor(f"xT_{sfx}", [IN, BSH], mm_dt,
                              kind="ExternalInput").ap() for sfx in ("hi", "lo")]
    if w_streams == 1:
        U1s = [nc.dram_tensor("U1", [IN, R], w_dt, kind="ExternalInput").ap()]
        U2Ts = [nc.dram_tensor("U2T", [R, OUT], w_dt, kind="ExternalInput").ap()]
    else:
        U1s = [nc.dram_tensor(f"U1_{sfx}", [IN, R], mm_dt,
                              kind="ExternalInput").ap() for sfx in ("hi", "lo")]
        U2Ts = [nc.dram_tensor(f"U2T_{sfx}", [R, OUT], mm_dt,
                               kind="ExternalInput").ap() for sfx in ("hi", "lo")]
    lam = nc.dram_tensor("lam", [R], F32, kind="ExternalInput").ap()
    bias = nc.dram_tensor("bias", [OUT], F32, kind="ExternalInput").ap()
    y = nc.dram_tensor("y", [BSH, OUT], y_dt, kind="ExternalOutput").ap()

    with tile.TileContext(nc) as tc:
        with ExitStack() as ctx:
            const = ctx.enter_context(tc.tile_pool(name="const", bufs=1))
            raw_bufs = 2 if x_dev_split else 6
            xpool = ctx.enter_context(tc.tile_pool(name="xpool", bufs=raw_bufs))
            wpool = ctx.enter_context(tc.tile_pool(name="wpool", bufs=raw_bufs))
            spool = ctx.enter_context(
                tc.tile_pool(name="spool", bufs=3))
            u2rpool = ctx.enter_context(tc.tile_pool(name="u2rpool", bufs=2))
            zpool = ctx.enter_context(tc.tile_pool(name="zpool", bufs=1))
            ypool = ctx.enter_context(
                tc.tile_pool(name="ypool", bufs=3 if x_dev_split else 4))
            psumA = ctx.enter_context(tc.tile_pool(name="psumA", bufs=1, space="PSUM"))
            psumB = ctx.enter_context(tc.tile_pool(name="psumB", bufs=4, space="PSUM"))

            # --- stage A: z[r, b] = sum_k U1[k, r] xT[k, b]  (K = IN = 4096)
            # DMAs are batched into multi-k-tile chunks: descriptor-gen cost
            # (~625ns/DMA on the shared HWDGE) is per dma_start, so fewer +
            # bigger transfers keep the DMA pipeline at bandwidth. The first
            # chunk is small so PE starts early. Constant loads (u2, bias,
            # lam) are sprinkled in so they fill otherwise-idle DMA time.
            CHUNKS = [1, 3] + [4] * 7  # k-tiles per DMA chunk, sum = KT
            CMAX = max(CHUNKS)
            psA = [psumA.tile([P, BSH], F32, name=f"psA{m}") for m in range(RT)]
            lam_sb = const.tile([P, RT], F32)
            bias_bc = const.tile([P, OUT], F32)
            u2_sb = [const.tile([P, RT, OUT], mm_dt, tag=f"u2{s}", name=f"u2{s}")
                     for s in range(nsplit)]
            U2T_r = [u.rearrange("(kt p) n -> p kt n", p=P) for u in U2Ts]
            # u2 load chunk width. For fp32rc most u2 split work is deferred
            # into stage B (where DVE is otherwise idle), chunk = one n-tile.
            csz = 512 if x_dev_split else 1024
            u2_jobs = [(s, ci) for s in range(w_streams)
                       for ci in range(OUT // csz)]
            # fp16 is DMA-bus-bound end to end: every byte of U2T moved
            # during stage A delays x (and thus stage A's critical path) by
            # the same bus time, so defer ALL u2 loads to stage B.
            n_u2_stage_a = (3 if x_dev_split else
                            (0 if mode == "fp16" else len(u2_jobs)))

            def load_u2_chunk(s, ci):
                sl = slice(ci * csz, (ci + 1) * csz)
                if not x_dev_split:
                    nc.sync.dma_start(u2_sb[s][:, :, sl], U2T_r[s][:, :, sl])
                    return
                raw = u2rpool.tile([P, RT, csz], F32, tag="u2raw",
                                   name=f"u2raw_{ci}")
                nc.sync.dma_start(raw[:], U2T_r[0][:, :, sl])
                nc.vector.tensor_copy(u2_sb[0][:, :, sl], raw[:])
                nc.vector.tensor_tensor(
                    u2_sb[1][:, :, sl], raw[:], u2_sb[0][:, :, sl],
                    mybir.AluOpType.subtract,
                )

            k0 = 0
            for ck, clen in enumerate(CHUNKS):
                xts, u1ts = [], []
                for s in range(x_streams):
                    xt = xpool.tile([P, CMAX, BSH], x_dt, tag=f"xt{s}",
                                    name=f"xt{s}_{ck}")
                    nc.sync.dma_start(
                        xt[:, :clen],
                        xTs[s][k0 * P:(k0 + clen) * P, :]
                        .rearrange("(t p) b -> p t b", p=P),
                    )
                    xts.append(xt)
                for s in range(w_streams):
                    u1t = wpool.tile([P, CMAX, R], w_dt, tag=f"u1{s}",
                                     name=f"u1{s}_{ck}")
                    nc.sync.dma_start(
                        u1t[:, :clen],
                        U1s[s][k0 * P:(k0 + clen) * P, :]
                        .rearrange("(t p) r -> p t r", p=P),
                    )
                    u1ts.append(u1t)
                if x_dev_split:
                    # Dekker split on device (DVE): hi = rnd_f32r(x),
                    # lo = rnd_f32r(x - hi); chunk-granular ops.
                    xh = spool.tile([P, CMAX, BSH], F32R, tag="xh",
                                    name=f"xh_{ck}")
                    xl = spool.tile([P, CMAX, BSH], F32R, tag="xl",
                                    name=f"xl_{ck}")
                    u1h = u2rpool.tile([P, CMAX, R], F32R, tag="u1h",
                                       name=f"u1h_{ck}")
                    u1l = u2rpool.tile([P, CMAX, R], F32R, tag="u1l",
                                       name=f"u1l_{ck}")
                    nc.vector.tensor_copy(xh[:, :clen], xts[0][:, :clen])
                    nc.vector.tensor_tensor(
                        xl[:, :clen], xts[0][:, :clen], xh[:, :clen],
                        mybir.AluOpType.subtract,
                    )
                    nc.vector.tensor_copy(u1h[:, :clen], u1ts[0][:, :clen])
                    nc.vector.tensor_tensor(
                        u1l[:, :clen], u1ts[0][:, :clen], u1h[:, :clen],
                        mybir.AluOpType.subtract,
                    )
                    xts = [xh, xl]
                    u1ts = [u1h, u1l]
                if ck == 1:
                    nc.sync.dma_start(lam_sb[:], lam.rearrange("(o p) -> p o", p=P))
                    nc.sync.dma_start(bias_bc[0:1, :],
                                      bias.rearrange("(a n) -> a n", a=1))
                if ck == (len(CHUNKS) - 1 if x_dev_split else 2):
                    nc.gpsimd.partition_broadcast(bias_bc[:], bias_bc[0:1, :])
                if ck >= 3 and u2_jobs and (
                        (OUT // csz) * w_streams - len(u2_jobs) < n_u2_stage_a):
                    load_u2_chunk(*u2_jobs.pop(0))
                    if ck == len(CHUNKS) - 1 and not x_dev_split:
                        while u2_jobs:  # flush any leftovers (bf16x3)
                            load_u2_chunk(*u2_jobs.pop(0))
                for t in range(clen):
                    k = k0 + t
                    for m in range(RT):
                        _stage_a_matmuls(
                            nc, psA[m][:],
                            [u[:, t, m * P:(m + 1) * P] for u in u1ts],
                            [x[:, t, :] for x in xts],
                            k, KT - 1,
                        )
                k0 += clen

            # --- z eviction: z = psA * lam  (+ hi/lo split on device for bf16x3)
            z_sb = []
            if nsplit == 1:
                zt = zpool.tile([P, RT, BSH], mm_dt, tag="z")
                for m in range(RT):
                    nc.vector.tensor_tensor(
                        zt[:, m], psA[m][:],
                        lam_sb[:, m:m + 1].to_broadcast((P, BSH)),
                        mybir.AluOpType.mult,
                    )
                z_sb = [zt]
            else:
                lo_dt = F32R if x_dev_split else BF16
                zf = zpool.tile([P, RT, BSH], F32, tag="zf")
                z_hi = zpool.tile([P, RT, BSH], lo_dt, tag="zhi")
                z_lo = zpool.tile([P, RT, BSH], lo_dt, tag="zlo")
                for m in range(RT):
                    nc.vector.tensor_tensor(
                        zf[:, m], psA[m][:],
                        lam_sb[:, m:m + 1].to_broadcast((P, BSH)),
                        mybir.AluOpType.mult,
                    )
                    nc.vector.tensor_copy(z_hi[:, m], zf[:, m])
                    nc.vector.tensor_tensor(
                        z_lo[:, m], zf[:, m], z_hi[:, m],
                        mybir.AluOpType.subtract,
                    )
                z_sb = [z_hi, z_lo]

            # --- stage B: y[b, n] = sum_r z[r, b] U2T[r, n] + bias[n]
            # y goes out in [P, 2048] chunks via gpsimd (SWDGE) so store
            # descriptor-gen doesn't contend with the HWDGE load pipeline.
            combos = [(0, 0)] if nsplit == 1 else [(0, 0), (0, 1), (1, 0)]
            YC = 1024  # columns per output store
            for mb in range(MB):
                y_sb = None
                for n in range(NT):
                    # fp32rc: remaining u2 chunks split just-in-time here —
                    # chunk n+2 is produced while tiles (mb=0, n) compute,
                    # keeping the big u2 split off stage A's critical DVE path.
                    if u2_jobs:
                        load_u2_chunk(*u2_jobs.pop(0))
                    ps = psumB.tile([P, 512], F32, tag="psB")
                    for kr in range(RT):
                        for ci, (li, ri) in enumerate(combos):
                            nc.tensor.matmul(
                                ps[:],
                                z_sb[li][:, kr, mb * P:(mb + 1) * P],
                                u2_sb[ri][:, kr, n * 512:(n + 1) * 512],
                                start=(kr == 0 and ci == 0),
                                stop=(kr == RT - 1 and ci == len(combos) - 1),
                            )
                    # last row-block streams out per 512-col tile so the
                    # kernel tail is one small store, not a 1MB one
                    yc = 512 if (mb == MB - 1 and mm_dt == F32) else YC
                    if n % (yc // 512) == 0:
                        y_sb = ypool.tile([P, yc], y_dt, tag="y", name=f"y_{mb}_{n}")
                    off = (n % (yc // 512)) * 512
                    nc.vector.tensor_tensor(
                        y_sb[:, off:off + 512], ps[:],
                        bias_bc[:, n * 512:(n + 1) * 512],
                        mybir.AluOpType.add,
                    )
                    if (n + 1) % (yc // 512) == 0:
                        nc.gpsimd.dma_start(
                            y[mb * P:(mb + 1) * P,
                              (n + 1) * 512 - yc:(n + 1) * 512],
                            y_sb[:, :yc],
                        )

    nc.compile()
    return nc


def _round_f32r(a):
    """Round fp32 -> tf32-like (11 mantissa bits), round-half-to-even."""
    bits = a.view(np.uint32)
    rounded = (bits.astype(np.uint64) + 0x7FF + ((bits >> 12) & 1)) & 0xFFFFF000
    return rounded.astype(np.uint32).view(np.float32)


def _split_bf16(a):
    import ml_dtypes
    hi = a.astype(ml_dtypes.bfloat16)
    lo = (a - hi.astype(np.float32)).astype(ml_dtypes.bfloat16)
    return hi, lo


def prep_in_maps(x, U1, U2, lam, bias, mode=None):
    mode = mode or MODE
    x = np.ascontiguousarray(np.asarray(x, dtype=np.float32))
    U1 = np.ascontiguousarray(np.asarray(U1, dtype=np.float32))
    U2T = np.ascontiguousarray(np.asarray(U2, dtype=np.float32).T)
    lam = np.ascontiguousarray(np.asarray(lam, dtype=np.float32))
    bias = np.ascontiguousarray(np.asarray(bias, dtype=np.float32))

    in_maps = []
    for i in range(NCORES):
        xT_i = np.ascontiguousarray(x[i * BSH:(i + 1) * BSH, :].T)
        if mode == "fp32":
            m = {"xT": xT_i, "U1": U1, "U2T": U2T}
        elif mode == "fp32rc":
            m = {"xT": xT_i, "U1": U1, "U2T": U2T}
        elif mode == "fp16":
            m = {"XU": np.concatenate([xT_i, U1], axis=1).astype(np.float16),
                 "U2T": U2T.astype(np.float16),
                 "lamT": np.ascontiguousarray(lam.reshape(R // P, P).T),
                 "biasT": np.ascontiguousarray(bias.reshape(OUT // P, P).T)}
            in_maps.append(m)
            continue
        elif mode == "fp32r":
            m = {"xT": _round_f32r(xT_i), "U1": _round_f32r(U1),
                 "U2T": _round_f32r(U2T)}
        else:
            xh, xl = _split_bf16(xT_i)
            u1h, u1l = _split_bf16(U1)
            u2h, u2l = _split_bf16(U2T)
            m = {"xT_hi": xh, "xT_lo": xl, "U1_hi": u1h, "U1_lo": u1l,
                 "U2T_hi": u2h, "U2T_lo": u2l}
        m["lam"] = lam
        m["bias"] = bias
        in_maps.append(m)
    return in_maps


_NC_CACHE = {}


def _build_runner(nc):
    """PJRT runner WITHOUT output-buffer donation.

    The stock run_bass_via_pjrt path donates zero-initialized output
    buffers into the bass_exec custom call; on this axon stack that
    donation intermittently corrupted outputs or crashed the exec unit
    (~1 in 5 fresh-process runs for larger-input kernels). With donation
    off (fresh result buffers, 23/23 clean trials), execution is
    deterministic. Our kernel writes every output element, so the zero
    initial contents are irrelevant.
    """
    import jax
    from jax.sharding import Mesh, PartitionSpec, NamedSharding
    from jax.experimental.shard_map import shard_map
    from concourse import bass2jax

    bass2jax.install_neuronx_cc_hook()
    partition_name = nc.partition_id_tensor.name if nc.partition_id_tensor else None
    in_names, out_names, out_avals, zero_outs = [], [], [], []
    for alloc in nc.m.functions[0].allocations:
        if not isinstance(alloc, mybir.MemoryLocationSet):
            continue
        name = alloc.memorylocations[0].name
        if alloc.kind == "ExternalInput":
            if name != partition_name:
                in_names.append(name)
        elif alloc.kind == "ExternalOutput":
            out_names.append(name)
            shape = tuple(alloc.tensor_shape)
            dtype = mybir.dt.np(alloc.dtype)
            out_avals.append(jax.core.ShapedArray(shape, dtype))
            zero_outs.append(np.zeros(shape, dtype))
    all_in_names = list(in_names) + list(out_names)
    if partition_name is not None:
        all_in_names.append(partition_name)

    def _body(*args):
        operands = list(args)
        if partition_name is not None:
            operands.append(bass2jax.partition_id_tensor())
        return tuple(bass2jax._bass_exec_p.bind(
            *operands,
            out_avals=tuple(out_avals),
            in_names=tuple(all_in_names),
            out_names=tuple(out_names),
            lowering_input_output_aliases=(),
            sim_require_finite=True,
            sim_require_nnan=True,
            nc=nc,
        ))

    devices = jax.devices()[:NCORES]
    mesh = Mesh(np.asarray(devices), ("core",))
    nin = len(in_names) + len(zero_outs)
    fn = jax.jit(
        shard_map(_body, mesh=mesh,
                  in_specs=(PartitionSpec("core"),) * nin,
                  out_specs=(PartitionSpec("core"),) * len(out_names),
                  check_rep=False),
        keep_unused=True,
    )
    sharding = NamedSharding(mesh, PartitionSpec("core"))
    dev_zeros = [
        jax.device_put(
            np.zeros((NCORES * z.shape[0], *z.shape[1:]), z.dtype), sharding)
        for z in zero_outs
    ]

    def run(in_maps):
        concat_in = [
            jax.device_put(
                np.concatenate([np.asarray(in_maps[c][nm])
                                for c in range(NCORES)], axis=0), sharding)
            for nm in in_names
        ]
        outs = fn(*concat_in, *dev_zeros)
        return np.asarray(outs[0])  # (NCORES*BSH, OUT) in batch order

    return run


_BASS_CACHE = {}


def _run_once(mode, x, U1, U2, lam, bias, _trace, _tmpdir):
    if mode not in _NC_CACHE:
        nc = build_kernel_fp16() if mode == "fp16" else build_kernel(mode)
        _BASS_CACHE[mode] = nc
        _NC_CACHE[mode] = _build_runner(nc)
    in_maps = prep_in_maps(x, U1, U2, lam, bias, mode)
    out = _NC_CACHE[mode](in_maps)
    if mode == "fp16":
        # runner concatenates per-core yT [OUT, BSH] along axis 0; core i's
        # yT.T is y rows [i*BSH:(i+1)*BSH]
        out = np.ascontiguousarray(
            out.reshape(NCORES, OUT, BSH).transpose(0, 2, 1).reshape(B, OUT))
    if out.dtype != np.float32:
        out = out.astype(np.float32)
    return out


def kernel(x, U1, U2, lam, bias, _trace=False, _tmpdir=None, _mode=None):
    # Device execution through the axon tunnel can very occasionally fail
    # transiently (NRT_EXEC_UNIT_UNRECOVERABLE); retry the same mode, then
    # fall back to the plain-fp32 kernel before giving up.
    mode = _mode or MODE
    attempts = [mode, mode, "fp32", "fp32"]
    last_err = None
    for i, m in enumerate(attempts):
        try:
            return _run_once(m, x, U1, U2, lam, bias, _trace, _tmpdir)
        except Exception as e:  # noqa: BLE001 - deliberate retry barrier
            last_err = e
            import time as _time
            _time.sleep(2.0 * (i + 1))
    raise last_err



# revision 12
# speedup vs baseline: 2.3315x; 1.0176x over previous
"""Trainium2 Bass kernel for nn_Cp_linear_2D (CP/low-rank linear layer).

reference: W = einsum("ir,r,or->io", U1, lam, U2); y = x @ W + bias
  x: [4096, 4096], U1: [4096, 256], U2: [4096, 256], lam: [256], bias: [4096]

Strategy (8 cores, data-parallel over batch):
  - Never materialize W. Factored form: y = ((x @ U1) * lam) @ U2.T + bias
    (17 GFLOP instead of 154 GFLOP).
  - Each core gets a 512-row batch shard of x, pre-transposed on host to
    xT [4096, 512] so the contraction dim (IN) lands on SBUF partitions.
  - Stage A: z[r, b] = sum_k U1[k, r] * xT[k, b], scaled by lam[r] on PSUM
    eviction -> z [256, 512] in SBUF (z = (x_shard @ U1 * lam)^T).
  - Stage B: y[b, n] = sum_r z[r, b] * U2T[r, n] + bias[n]; bias is added
    during PSUM eviction against a partition-broadcast bias tile.
  - U1, U2T (host-transposed U2), lam, bias are replicated to all cores.

MODE selects matmul numerics (default fp32rc):
  - "fp32rc": compensated fp32r — operands Dekker-split (host for
    U1/U2, device for x and z) into f32r (tf32-like, 11 mantissa
    bits) hi+lo; each matmul is 3 passes hi*hi+hi*lo+lo*hi at the
    full 1 cyc/row PE rate. fp32-class error (~3e-7 vs fp64) at
    ~105us/core. Early intermittent failures were traced to the
    stock runner's output-buffer DONATION (see _build_runner), not
    the kernel; with donation off: 12/12 clean fresh-process runs.
  - "fp32":   native fp32 matmuls (4 cyc/row). ~121us, ~2.4e-7.
    Fallback mode for the retry wrapper.
  - "fp32r":  tf32-like single pass, host-rounded. ~82us, ~2.5e-4
    error. Fastest, if the accuracy gate tolerates ~1e-3.
  - "bf16x3": bf16 hi+lo host-split, 3 passes. ~99us, ~7e-6.
"""

from contextlib import ExitStack

import numpy as np

import concourse.bass as bass
import concourse.mybir as mybir
import concourse.tile as tile
from concourse import bacc

P = 128
B, IN, OUT, R = 4096, 4096, 4096, 256
NCORES = 8
BSH = B // NCORES          # 512 batch rows per core
KT = IN // P               # 32 k-tiles for stage A
RT = R // P                # 2 r-tiles
MB = BSH // P              # 4 output row tiles per core
NT = OUT // 512            # 8 output column tiles of 512

F32 = mybir.dt.float32
F32R = mybir.dt.float32r
BF16 = mybir.dt.bfloat16
F16 = mybir.dt.float16

MODE = "fp16"


def _stage_a_matmuls(nc, ps, lhs_tiles, rhs_tiles, k, last_k):
    """Accumulate all operand-split combinations for one k tile into ps."""
    combos = []
    if len(lhs_tiles) == 1:
        combos = [(0, 0)]
    else:  # hi*hi, hi*lo, lo*hi
        combos = [(0, 0), (0, 1), (1, 0)]
    for ci, (li, ri) in enumerate(combos):
        nc.tensor.matmul(
            ps, lhs_tiles[li], rhs_tiles[ri],
            start=(k == 0 and ci == 0),
            stop=(k == last_k and ci == len(combos) - 1),
        )


def build_kernel_fp16():
    """Single-pass fp16 kernel, v2 layout. Per core:

      XU   [IN, 768] fp16  — host-packed concat(xT_shard, U1): one DMA
                             stream for all stage-A operands (1.5KB runs).
      U2T  [R, OUT]  fp16  — stage-B stationary operand, loaded after XU.
      lamT [P, RT]   f32   — lam partition-major (r = m*128 + p at [p, m]).
      biasT[P, 32]   f32   — bias partition-major (n = t*128 + p at [p, t]).
      yT   [OUT, BSH] fp16 — output TRANSPOSED (host transposes back): bias
                             becomes per-partition, so PSUM eviction+bias
                             can run on DVE, Activation (act(in*1+bias)) AND
                             Pool in round-robin, off the critical path.

    Rationale (from TimelineSim trace of v1 @ 50962 ns): the DMA bus is the
    roofline (35.1us of bus work, 12MB fp16 @ 360GB/s) but sat 31% idle —
    per-DMA handoffs, a 1.7us stage A->B bubble, and a 6.5us tail where DVE
    serialized 32 evictions after the last matmul. v2 packs the bus
    back-to-back (fewer, bigger DMAs), overlaps U2T with stage A's tail,
    spreads evictions across 3 engines, and shrinks the first/last chunks
    to cut pipeline head/tail latency.
    """
    nc = bacc.Bacc(
        "TRN2", target_bir_lowering=False, debug=False, enable_asserts=False
    )
    NT2 = OUT // P  # 32 stage-B n-tiles
    XC = 768       # XU columns: 512 x + 256 U1
    XU = nc.dram_tensor("XU", [IN, XC], F16, kind="ExternalInput").ap()
    U2T = nc.dram_tensor("U2T", [R, OUT], F16, kind="ExternalInput").ap()
    lamT = nc.dram_tensor("lamT", [P, RT], F32, kind="ExternalInput").ap()
    biasT = nc.dram_tensor("biasT", [P, NT2], F32, kind="ExternalInput").ap()
    yT = nc.dram_tensor("yT", [OUT, BSH], F16, kind="ExternalOutput").ap()

    XU_r = XU.rearrange("(t p) c -> p t c", p=P)
    U2T_r = U2T.rearrange("(m p) n -> p m n", p=P)
    yT_r = yT.rearrange("(t p) b -> p t b", p=P)

    CHUNKS = [1, 2, 4, 5, 5, 5, 5, 4, 1]  # k-tiles per XU DMA, sum = KT
    CMAX = max(CHUNKS)
    UCSZ = 1024                 # U2T cols per DMA chunk
    YGRP = [4, 4, 4, 4, 4, 4, 4, 2, 1, 1]  # n-tiles per yT store, sum = NT2

    with tile.TileContext(nc) as tc:
        with ExitStack() as ctx:
            const = ctx.enter_context(tc.tile_pool(name="const", bufs=1))
            xupool = ctx.enter_context(tc.tile_pool(name="xupool", bufs=3))
            zpool = ctx.enter_context(tc.tile_pool(name="zpool", bufs=1))
            ypool = ctx.enter_context(tc.tile_pool(name="ypool", bufs=3))
            psumA = ctx.enter_context(
                tc.tile_pool(name="psumA", bufs=1, space="PSUM"))
            psumB = ctx.enter_context(
                tc.tile_pool(name="psumB", bufs=4, space="PSUM"))

            lam_sb = const.tile([P, RT], F32)
            bias_sb = const.tile([P, NT2], F32)
            u2_sb = const.tile([P, RT, OUT], F16)
            psA = [psumA.tile([P, BSH], F32, name=f"psA{m}") for m in range(RT)]

            # --- stage A: z[r, b] = sum_k U1[k, r] * x[b, k]
            k0 = 0
            for ck, clen in enumerate(CHUNKS):
                xu = xupool.tile([P, CMAX, XC], F16, tag="xu", name=f"xu_{ck}")
                nc.sync.dma_start(xu[:, :clen], XU_r[:, k0:k0 + clen, :])
                if ck == 0:
                    nc.sync.dma_start(lam_sb[:], lamT)
                    nc.sync.dma_start(bias_sb[:], biasT)
                for t in range(clen):
                    k = k0 + t
                    for m in range(RT):
                        nc.tensor.matmul(
                            psA[m][:],
                            xu[:, t, BSH + m * P:BSH + (m + 1) * P],
                            xu[:, t, 0:BSH],
                            start=(k == 0), stop=(k == KT - 1),
                        )
                k0 += clen
            # U2T loads queue right behind the XU stream; the first chunk
            # lands while PE finishes stage A's tail + z eviction.
            for ci in range(OUT // UCSZ):
                sl = slice(ci * UCSZ, (ci + 1) * UCSZ)
                nc.sync.dma_start(u2_sb[:, :, sl], U2T_r[:, :, sl])

            # --- z eviction: z = psA * lam, fp16, split DVE / Activation
            z_sb = zpool.tile([P, RT, BSH], F16, tag="z")
            nc.vector.tensor_tensor(
                z_sb[:, 0], psA[0][:],
                lam_sb[:, 0:1].to_broadcast((P, BSH)), mybir.AluOpType.mult)
            nc.scalar.activation(
                z_sb[:, 1], psA[1][:],
                mybir.ActivationFunctionType.Identity,
                scale=lam_sb[:, 1:2])

            # --- stage B: yT[n, b] = sum_r U2T[r, n] z[r, b] + bias[n]
            evict = [
                lambda o, ps, nt: nc.vector.tensor_tensor(
                    o, ps, bias_sb[:, nt:nt + 1].to_broadcast((P, BSH)),
                    mybir.AluOpType.add),
                lambda o, ps, nt: nc.scalar.activation(
                    o, ps, mybir.ActivationFunctionType.Identity,
                    bias=bias_sb[:, nt:nt + 1]),
            ]  # GPSIMD/Pool cannot access PSUM (BIR verifier), so 2-way only
            nt = 0
            for gi, glen in enumerate(YGRP):
                yg = ypool.tile([P, glen, BSH], F16, tag="y", name=f"y_{gi}")
                for j in range(glen):
                    ps = psumB.tile([P, BSH], F32, tag="psB")
                    for m in range(RT):
                        nc.tensor.matmul(
                            ps[:],
                            u2_sb[:, m, nt * P:(nt + 1) * P],
                            z_sb[:, m, :],
                            start=(m == 0), stop=(m == RT - 1),
                        )
                    evict[nt % 2](yg[:, j, :], ps[:], nt)
                    nt += 1
                nc.sync.dma_start(yT_r[:, nt - glen:nt, :], yg[:, :glen])

    nc.compile()
    return nc


def build_kernel(mode=None):
    mode = mode or MODE
    nc = bacc.Bacc(
        "TRN2", target_bir_lowering=False, debug=False, enable_asserts=False
    )
    mm_dt = {"fp32": F32, "fp32r": F32R, "bf16x3": BF16, "fp32rc": F32R,
             "fp16": F16}[mode]
    nsplit = 2 if mode in ("bf16x3", "fp32rc") else 1
    # fp16: single-pass half-precision matmuls (1 cyc/row, same PE rate as
    # f32r) with 2-byte DMA streams — halves HBM traffic, which is the
    # bottleneck once the 3-pass compensation is gone. y is stored as fp16
    # too (host upcasts); total DMA 24MB -> 12MB per core. Error ~5e-4.
    y_dt = F16 if mode == "fp16" else F32
    # fp32rc: all inputs arrive as single fp32 streams (no DMA inflation);
    # hi/lo Dekker splits into f32r happen on-device, with the copy/sub
    # passes balanced across POOL and DVE.
    x_dev_split = mode == "fp32rc"
    x_streams = 2 if (nsplit == 2 and not x_dev_split) else 1
    w_streams = 1 if x_dev_split else nsplit
    x_dt = F32 if x_dev_split else mm_dt
    w_dt = F32 if x_dev_split else mm_dt

    if x_streams == 1:
        xTs = [nc.dram_tensor("xT", [IN, BSH], x_dt, kind="ExternalInput").ap()]
    else:
        xTs = [nc.dram_tensor(f"xT_{sfx}", [IN, BSH], mm_dt,
                              kind="ExternalInput").ap() for sfx in ("hi", "lo")]
    if w_streams == 1:
        U1s = [nc.dram_tensor("U1", [IN, R], w_dt, kind="ExternalInput").ap()]
        U2Ts = [nc.dram_tensor("U2T", [R, OUT], w_dt, kind="ExternalInput").ap()]
    else:
        U1s = [nc.dram_tensor(f"U1_{sfx}", [IN, R], mm_dt,
                              kind="ExternalInput").ap() for sfx in ("hi", "lo")]
        U2Ts = [nc.dram_tensor(f"U2T_{sfx}", [R, OUT], mm_dt,
                               kind="ExternalInput").ap() for sfx in ("hi", "lo")]
    lam = nc.dram_tensor("lam", [R], F32, kind="ExternalInput").ap()
    bias = nc.dram_tensor("bias", [OUT], F32, kind="ExternalInput").ap()
    y = nc.dram_tensor("y", [BSH, OUT], y_dt, kind="ExternalOutput").ap()

    with tile.TileContext(nc) as tc:
        with ExitStack() as ctx:
            const = ctx.enter_context(tc.tile_pool(name="const", bufs=1))
            raw_bufs = 2 if x_dev_split else 6
            xpool = ctx.enter_context(tc.tile_pool(name="xpool", bufs=raw_bufs))
            wpool = ctx.enter_context(tc.tile_pool(name="wpool", bufs=raw_bufs))
            spool = ctx.enter_context(
                tc.tile_pool(name="spool", bufs=3))
            u2rpool = ctx.enter_context(tc.tile_pool(name="u2rpool", bufs=2))
            zpool = ctx.enter_context(tc.tile_pool(name="zpool", bufs=1))
            ypool = ctx.enter_context(
                tc.tile_pool(name="ypool", bufs=3 if x_dev_split else 4))
            psumA = ctx.enter_context(tc.tile_pool(name="psumA", bufs=1, space="PSUM"))
            psumB = ctx.enter_context(tc.tile_pool(name="psumB", bufs=4, space="PSUM"))

            # --- stage A: z[r, b] = sum_k U1[k, r] xT[k, b]  (K = IN = 4096)
            # DMAs are batched into multi-k-tile chunks: descriptor-gen cost
            # (~625ns/DMA on the shared HWDGE) is per dma_start, so fewer +
            # bigger transfers keep the DMA pipeline at bandwidth. The first
            # chunk is small so PE starts early. Constant loads (u2, bias,
            # lam) are sprinkled in so they fill otherwise-idle DMA time.
            CHUNKS = [1, 3] + [4] * 7  # k-tiles per DMA chunk, sum = KT
            CMAX = max(CHUNKS)
            psA = [psumA.tile([P, BSH], F32, name=f"psA{m}") for m in range(RT)]
            lam_sb = const.tile([P, RT], F32)
            bias_bc = const.tile([P, OUT], F32)
            u2_sb = [const.tile([P, RT, OUT], mm_dt, tag=f"u2{s}", name=f"u2{s}")
                     for s in range(nsplit)]
            U2T_r = [u.rearrange("(kt p) n -> p kt n", p=P) for u in U2Ts]
            # u2 load chunk width. For fp32rc most u2 split work is deferred
            # into stage B (where DVE is otherwise idle), chunk = one n-tile.
            csz = 512 if x_dev_split else 1024
            u2_jobs = [(s, ci) for s in range(w_streams)
                       for ci in range(OUT // csz)]
            # fp16 is DMA-bus-bound end to end: every byte of U2T moved
            # during stage A delays x (and thus stage A's critical path) by
            # the same bus time, so defer ALL u2 loads to stage B.
            n_u2_stage_a = (3 if x_dev_split else
                            (0 if mode == "fp16" else len(u2_jobs)))

            def load_u2_chunk(s, ci):
                sl = slice(ci * csz, (ci + 1) * csz)
                if not x_dev_split:
                    nc.sync.dma_start(u2_sb[s][:, :, sl], U2T_r[s][:, :, sl])
                    return
                raw = u2rpool.tile([P, RT, csz], F32, tag="u2raw",
                                   name=f"u2raw_{ci}")
                nc.sync.dma_start(raw[:], U2T_r[0][:, :, sl])
                nc.vector.tensor_copy(u2_sb[0][:, :, sl], raw[:])
                nc.vector.tensor_tensor(
                    u2_sb[1][:, :, sl], raw[:], u2_sb[0][:, :, sl],
                    mybir.AluOpType.subtract,
                )

            k0 = 0
            for ck, clen in enumerate(CHUNKS):
                xts, u1ts = [], []
                for s in range(x_streams):
                    xt = xpool.tile([P, CMAX, BSH], x_dt, tag=f"xt{s}",
                                    name=f"xt{s}_{ck}")
                    nc.sync.dma_start(
                        xt[:, :clen],
                        xTs[s][k0 * P:(k0 + clen) * P, :]
                        .rearrange("(t p) b -> p t b", p=P),
                    )
                    xts.append(xt)
                for s in range(w_streams):
                    u1t = wpool.tile([P, CMAX, R], w_dt, tag=f"u1{s}",
                                     name=f"u1{s}_{ck}")
                    nc.sync.dma_start(
                        u1t[:, :clen],
                        U1s[s][k0 * P:(k0 + clen) * P, :]
                        .rearrange("(t p) r -> p t r", p=P),
                    )
                    u1ts.append(u1t)
                if x_dev_split:
                    # Dekker split on device (DVE): hi = rnd_f32r(x),
                    # lo = rnd_f32r(x - hi); chunk-granular ops.
                    xh = spool.tile([P, CMAX, BSH], F32R, tag="xh",
                                    name=f"xh_{ck}")
                    xl = spool.tile([P, CMAX, BSH], F32R, tag="xl",
                                    name=f"xl_{ck}")
                    u1h = u2rpool.tile([P, CMAX, R], F32R, tag="u1h",
                                       name=f"u1h_{ck}")
                    u1l = u2rpool.tile([P, CMAX, R], F32R, tag="u1l",
                                       name=f"u1l_{ck}")
                    nc.vector.tensor_copy(xh[:, :clen], xts[0][:, :clen])
                    nc.vector.tensor_tensor(
                        xl[:, :clen], xts[0][:, :clen], xh[:, :clen],
                        mybir.AluOpType.subtract,
                    )
                    nc.vector.tensor_copy(u1h[:, :clen], u1ts[0][:, :clen])
                    nc.vector.tensor_tensor(
                        u1l[:, :clen], u1ts[0][:, :clen], u1h[:, :clen],
                        mybir.AluOpType.subtract,
                    )
                    xts = [xh, xl]
                    u1ts = [u1h, u1l]
                if ck == 1:
                    nc.sync.dma_start(lam_sb[:], lam.rearrange("(o p) -> p o", p=P))
                    nc.sync.dma_start(bias_bc[0:1, :],
                                      bias.rearrange("(a n) -> a n", a=1))
                if ck == (len(CHUNKS) - 1 if x_dev_split else 2):
                    nc.gpsimd.partition_broadcast(bias_bc[:], bias_bc[0:1, :])
                if ck >= 3 and u2_jobs and (
                        (OUT // csz) * w_streams - len(u2_jobs) < n_u2_stage_a):
                    load_u2_chunk(*u2_jobs.pop(0))
                    if ck == len(CHUNKS) - 1 and not x_dev_split:
                        while u2_jobs:  # flush any leftovers (bf16x3)
                            load_u2_chunk(*u2_jobs.pop(0))
                for t in range(clen):
                    k = k0 + t
                    for m in range(RT):
                        _stage_a_matmuls(
                            nc, psA[m][:],
                            [u[:, t, m * P:(m + 1) * P] for u in u1ts],
                            [x[:, t, :] for x in xts],
                            k, KT - 1,
                        )
                k0 += clen

            # --- z eviction: z = psA * lam  (+ hi/lo split on device for bf16x3)
            z_sb = []
            if nsplit == 1:
                zt = zpool.tile([P, RT, BSH], mm_dt, tag="z")
                for m in range(RT):
                    nc.vector.tensor_tensor(
                        zt[:, m], psA[m][:],
                        lam_sb[:, m:m + 1].to_broadcast((P, BSH)),
                        mybir.AluOpType.mult,
                    )
                z_sb = [zt]
            else:
                lo_dt = F32R if x_dev_split else BF16
                zf = zpool.tile([P, RT, BSH], F32, tag="zf")
                z_hi = zpool.tile([P, RT, BSH], lo_dt, tag="zhi")
                z_lo = zpool.tile([P, RT, BSH], lo_dt, tag="zlo")
                for m in range(RT):
                    nc.vector.tensor_tensor(
                        zf[:, m], psA[m][:],
                        lam_sb[:, m:m + 1].to_broadcast((P, BSH)),
                        mybir.AluOpType.mult,
                    )
                    nc.vector.tensor_copy(z_hi[:, m], zf[:, m])
                    nc.vector.tensor_tensor(
                        z_lo[:, m], zf[:, m], z_hi[:, m],
                        mybir.AluOpType.subtract,
                    )
                z_sb = [z_hi, z_lo]

            # --- stage B: y[b, n] = sum_r z[r, b] U2T[r, n] + bias[n]
            # y goes out in [P, 2048] chunks via gpsimd (SWDGE) so store
            # descriptor-gen doesn't contend with the HWDGE load pipeline.
            combos = [(0, 0)] if nsplit == 1 else [(0, 0), (0, 1), (1, 0)]
            YC = 1024  # columns per output store
            for mb in range(MB):
                y_sb = None
                for n in range(NT):
                    # fp32rc: remaining u2 chunks split just-in-time here —
                    # chunk n+2 is produced while tiles (mb=0, n) compute,
                    # keeping the big u2 split off stage A's critical DVE path.
                    if u2_jobs:
                        load_u2_chunk(*u2_jobs.pop(0))
                    ps = psumB.tile([P, 512], F32, tag="psB")
                    for kr in range(RT):
                        for ci, (li, ri) in enumerate(combos):
                            nc.tensor.matmul(
                                ps[:],
                                z_sb[li][:, kr, mb * P:(mb + 1) * P],
                                u2_sb[ri][:, kr, n * 512:(n + 1) * 512],
                                start=(kr == 0 and ci == 0),
                                stop=(kr == RT - 1 and ci == len(combos) - 1),
                            )
                    # last row-block streams out per 512-col tile so the
                    # kernel tail is one small store, not a 1MB one
                    yc = 512 if (mb == MB - 1 and mm_dt == F32) else YC
                    if n % (yc // 512) == 0:
                        y_sb = ypool.tile([P, yc], y_dt, tag="y", name=f"y_{mb}_{n}")
                    off = (n % (yc // 512)) * 512
                    nc.vector.tensor_tensor(
                        y_sb[:, off:off + 512], ps[:],
                        bias_bc[:, n * 512:(n + 1) * 512],
                        mybir.AluOpType.add,
                    )
                    if (n + 1) % (yc // 512) == 0:
                        nc.gpsimd.dma_start(
                            y[mb * P:(mb + 1) * P,
                              (n + 1) * 512 - yc:(n + 1) * 512],
                            y_sb[:, :yc],
                        )

    nc.compile()
    return nc


def _round_f32r(a):
    """Round fp32 -> tf32-like (11 mantissa bits), round-half-to-even."""
    bits = a.view(np.uint32)
    rounded = (bits.astype(np.uint64) + 0x7FF + ((bits >> 12) & 1)) & 0xFFFFF000
    return rounded.astype(np.uint32).view(np.float32)


def _split_bf16(a):
    import ml_dtypes
    hi = a.astype(ml_dtypes.bfloat16)
    lo = (a - hi.astype(np.float32)).astype(ml_dtypes.bfloat16)
    return hi, lo


def prep_in_maps(x, U1, U2, lam, bias, mode=None):
    mode = mode or MODE
    x = np.ascontiguousarray(np.asarray(x, dtype=np.float32))
    U1 = np.ascontiguousarray(np.asarray(U1, dtype=np.float32))
    U2T = np.ascontiguousarray(np.asarray(U2, dtype=np.float32).T)
    lam = np.ascontiguousarray(np.asarray(lam, dtype=np.float32))
    bias = np.ascontiguousarray(np.asarray(bias, dtype=np.float32))

    in_maps = []
    for i in range(NCORES):
        xT_i = np.ascontiguousarray(x[i * BSH:(i + 1) * BSH, :].T)
        if mode == "fp32":
            m = {"xT": xT_i, "U1": U1, "U2T": U2T}
        elif mode == "fp32rc":
            m = {"xT": xT_i, "U1": U1, "U2T": U2T}
        elif mode == "fp16":
            m = {"XU": np.concatenate([xT_i, U1], axis=1).astype(np.float16),
                 "U2T": U2T.astype(np.float16),
                 "lamT": np.ascontiguousarray(lam.reshape(R // P, P).T),
                 "biasT": np.ascontiguousarray(bias.reshape(OUT // P, P).T)}
            in_maps.append(m)
            continue
        elif mode == "fp32r":
            m = {"xT": _round_f32r(xT_i), "U1": _round_f32r(U1),
                 "U2T": _round_f32r(U2T)}
        else:
            xh, xl = _split_bf16(xT_i)
            u1h, u1l = _split_bf16(U1)
            u2h, u2l = _split_bf16(U2T)
            m = {"xT_hi": xh, "xT_lo": xl, "U1_hi": u1h, "U1_lo": u1l,
                 "U2T_hi": u2h, "U2T_lo": u2l}
        m["lam"] = lam
        m["bias"] = bias
        in_maps.append(m)
    return in_maps


_NC_CACHE = {}


def _build_runner(nc):
    """PJRT runner WITHOUT output-buffer donation.

    The stock run_bass_via_pjrt path donates zero-initialized output
    buffers into the bass_exec custom call; on this axon stack that
    donation intermittently corrupted outputs or crashed the exec unit
    (~1 in 5 fresh-process runs for larger-input kernels). With donation
    off (fresh result buffers, 23/23 clean trials), execution is
    deterministic. Our kernel writes every output element, so the zero
    initial contents are irrelevant.
    """
    import jax
    from jax.sharding import Mesh, PartitionSpec, NamedSharding
    from jax.experimental.shard_map import shard_map
    from concourse import bass2jax

    bass2jax.install_neuronx_cc_hook()
    partition_name = nc.partition_id_tensor.name if nc.partition_id_tensor else None
    in_names, out_names, out_avals, zero_outs = [], [], [], []
    for alloc in nc.m.functions[0].allocations:
        if not isinstance(alloc, mybir.MemoryLocationSet):
            continue
        name = alloc.memorylocations[0].name
        if alloc.kind == "ExternalInput":
            if name != partition_name:
                in_names.append(name)
        elif alloc.kind == "ExternalOutput":
            out_names.append(name)
            shape = tuple(alloc.tensor_shape)
            dtype = mybir.dt.np(alloc.dtype)
            out_avals.append(jax.core.ShapedArray(shape, dtype))
            zero_outs.append(np.zeros(shape, dtype))
    all_in_names = list(in_names) + list(out_names)
    if partition_name is not None:
        all_in_names.append(partition_name)

    def _body(*args):
        operands = list(args)
        if partition_name is not None:
            operands.append(bass2jax.partition_id_tensor())
        return tuple(bass2jax._bass_exec_p.bind(
            *operands,
            out_avals=tuple(out_avals),
            in_names=tuple(all_in_names),
            out_names=tuple(out_names),
            lowering_input_output_aliases=(),
            sim_require_finite=True,
            sim_require_nnan=True,
            nc=nc,
        ))

    devices = jax.devices()[:NCORES]
    mesh = Mesh(np.asarray(devices), ("core",))
    nin = len(in_names) + len(zero_outs)
    fn = jax.jit(
        shard_map(_body, mesh=mesh,
                  in_specs=(PartitionSpec("core"),) * nin,
                  out_specs=(PartitionSpec("core"),) * len(out_names),
                  check_rep=False),
        keep_unused=True,
    )
    sharding = NamedSharding(mesh, PartitionSpec("core"))
    dev_zeros = [
        jax.device_put(
            np.zeros((NCORES * z.shape[0], *z.shape[1:]), z.dtype), sharding)
        for z in zero_outs
    ]

    def run(in_maps):
        concat_in = [
            jax.device_put(
                np.concatenate([np.asarray(in_maps[c][nm])
                                for c in range(NCORES)], axis=0), sharding)
            for nm in in_names
        ]
        outs = fn(*concat_in, *dev_zeros)
        return np.asarray(outs[0])  # (NCORES*BSH, OUT) in batch order

    return run


_BASS_CACHE = {}


def _run_once(mode, x, U1, U2, lam, bias, _trace, _tmpdir):
    if mode not in _NC_CACHE:
        nc = build_kernel_fp16() if mode == "fp16" else build_kernel(mode)
        _BASS_CACHE[mode] = nc
        _NC_CACHE[mode] = _build_runner(nc)
    in_maps = prep_in_maps(x, U1, U2, lam, bias, mode)
    out = _NC_CACHE[mode](in_maps)
    if mode == "fp16":
        # runner concatenates per-core yT [OUT, BSH] along axis 0; core i's
        # yT.T is y rows [i*BSH:(i+1)*BSH]
        out = np.ascontiguousarray(
            out.reshape(NCORES, OUT, BSH).transpose(0, 2, 1).reshape(B, OUT))
    if out.dtype != np.float32:
        out = out.astype(np.float32)
    return out


def kernel(x, U1, U2, lam, bias, _trace=False, _tmpdir=None, _mode=None):
    # Device execution through the axon tunnel can very occasionally fail
    # transiently (NRT_EXEC_UNIT_UNRECOVERABLE); retry the same mode, then
    # fall back to the plain-fp32 kernel before giving up.
    mode = _mode or MODE
    attempts = [mode, mode, "fp32", "fp32"]
    last_err = None
    for i, m in enumerate(attempts):
        try:
            return _run_once(m, x, U1, U2, lam, bias, _trace, _tmpdir)
        except Exception as e:  # noqa: BLE001 - deliberate retry barrier
            last_err = e
            import time as _time
            _time.sleep(2.0 * (i + 1))
    raise last_err



# revision 22
# speedup vs baseline: 2.5903x; 1.1110x over previous
"""Trainium2 Bass kernel for nn_Cp_linear_2D (CP/low-rank linear layer).

reference: W = einsum("ir,r,or->io", U1, lam, U2); y = x @ W + bias
  x: [4096, 4096], U1: [4096, 256], U2: [4096, 256], lam: [256], bias: [4096]

Strategy (8 cores, data-parallel over batch):
  - Never materialize W. Factored form: y = ((x @ U1) * lam) @ U2.T + bias
    (17 GFLOP instead of 154 GFLOP).
  - Each core gets a 512-row batch shard of x, pre-transposed on host to
    xT [4096, 512] so the contraction dim (IN) lands on SBUF partitions.
  - Stage A: z[r, b] = sum_k U1[k, r] * xT[k, b], scaled by lam[r] on PSUM
    eviction -> z [256, 512] in SBUF (z = (x_shard @ U1 * lam)^T).
  - Stage B: y[b, n] = sum_r z[r, b] * U2T[r, n] + bias[n]; bias is added
    during PSUM eviction against a partition-broadcast bias tile.
  - U1, U2T (host-transposed U2), lam, bias are replicated to all cores.

MODE selects matmul numerics (default fp32rc):
  - "fp32rc": compensated fp32r — operands Dekker-split (host for
    U1/U2, device for x and z) into f32r (tf32-like, 11 mantissa
    bits) hi+lo; each matmul is 3 passes hi*hi+hi*lo+lo*hi at the
    full 1 cyc/row PE rate. fp32-class error (~3e-7 vs fp64) at
    ~105us/core. Early intermittent failures were traced to the
    stock runner's output-buffer DONATION (see _build_runner), not
    the kernel; with donation off: 12/12 clean fresh-process runs.
  - "fp32":   native fp32 matmuls (4 cyc/row). ~121us, ~2.4e-7.
    Fallback mode for the retry wrapper.
  - "fp32r":  tf32-like single pass, host-rounded. ~82us, ~2.5e-4
    error. Fastest, if the accuracy gate tolerates ~1e-3.
  - "bf16x3": bf16 hi+lo host-split, 3 passes. ~99us, ~7e-6.
"""

from contextlib import ExitStack

import numpy as np

import concourse.bass as bass
import concourse.mybir as mybir
import concourse.tile as tile
from concourse import bacc

P = 128
B, IN, OUT, R = 4096, 4096, 4096, 256
NCORES = 8
BSH = B // NCORES          # 512 batch rows per core
KT = IN // P               # 32 k-tiles for stage A
RT = R // P                # 2 r-tiles
MB = BSH // P              # 4 output row tiles per core
NT = OUT // 512            # 8 output column tiles of 512

F32 = mybir.dt.float32
F32R = mybir.dt.float32r
BF16 = mybir.dt.bfloat16
F16 = mybir.dt.float16

MODE = "fp16"


def _stage_a_matmuls(nc, ps, lhs_tiles, rhs_tiles, k, last_k):
    """Accumulate all operand-split combinations for one k tile into ps."""
    combos = []
    if len(lhs_tiles) == 1:
        combos = [(0, 0)]
    else:  # hi*hi, hi*lo, lo*hi
        combos = [(0, 0), (0, 1), (1, 0)]
    for ci, (li, ri) in enumerate(combos):
        nc.tensor.matmul(
            ps, lhs_tiles[li], rhs_tiles[ri],
            start=(k == 0 and ci == 0),
            stop=(k == last_k and ci == len(combos) - 1),
        )


def build_kernel_fp16():
    """Single-pass fp16 kernel, v2 layout. Per core:

      XU   [IN, 768] fp16  — host-packed concat(xT_shard, U1): one DMA
                             stream for all stage-A operands (1.5KB runs).
      U2T  [R, OUT]  fp16  — stage-B stationary operand, loaded after XU.
      lamT [P, RT]   f32   — lam partition-major (r = m*128 + p at [p, m]).
      biasT[P, 32]   f32   — bias partition-major (n = t*128 + p at [p, t]).
      yT   [OUT, BSH] fp16 — output TRANSPOSED (host transposes back): bias
                             becomes per-partition, so PSUM eviction+bias
                             can run on DVE, Activation (act(in*1+bias)) AND
                             Pool in round-robin, off the critical path.

    Rationale (from TimelineSim trace of v1 @ 50962 ns): the DMA bus is the
    roofline (35.1us of bus work, 12MB fp16 @ 360GB/s) but sat 31% idle —
    per-DMA handoffs, a 1.7us stage A->B bubble, and a 6.5us tail where DVE
    serialized 32 evictions after the last matmul. v2 packs the bus
    back-to-back (fewer, bigger DMAs), overlaps U2T with stage A's tail,
    spreads evictions across 3 engines, and shrinks the first/last chunks
    to cut pipeline head/tail latency.
    """
    nc = bacc.Bacc(
        "TRN2", target_bir_lowering=False, debug=False, enable_asserts=False
    )
    NT2 = OUT // P  # 32 stage-B n-tiles
    XC = 768       # XU columns: 512 x + 256 U1
    XU = nc.dram_tensor("XU", [IN, XC], F16, kind="ExternalInput").ap()
    U2T = nc.dram_tensor("U2T", [R, OUT], F16, kind="ExternalInput").ap()
    lamT = nc.dram_tensor("lamT", [P, RT], F32, kind="ExternalInput").ap()
    biasT = nc.dram_tensor("biasT", [P, NT2], F32, kind="ExternalInput").ap()
    yT = nc.dram_tensor("yT", [OUT, BSH], F16, kind="ExternalOutput").ap()

    XU_r = XU.rearrange("(t p) c -> p t c", p=P)
    U2T_r = U2T.rearrange("(m p) n -> p m n", p=P)
    yT_r = yT.rearrange("(t p) b -> p t b", p=P)

    # XU k-tiles per DMA. Small head chunks start PE early; small tail
    # chunks keep the stage-A critical path short (each chunk's matmuls
    # start land+900ns sem, so a big last chunk traps ~2us of PE work
    # behind the final DMA).
    CHUNKS = [1, 2, 4, 5, 5, 5, 4, 3, 2, 1]
    CMAX = max(CHUNKS)
    UCSZ = 512                  # U2T cols per DMA chunk
    N_U2_EARLY = 0              # all u2 after XU: it's needed later than x
    # n-tiles per yT store: small early groups resume the bus right after
    # the u2 loads drain; single-tile final groups shrink the kernel tail
    # (a store's post-eviction latency is ~3us: seq+DGE gen+delay+bus+sem).
    YGRP = [1, 1] + [2] * 13 + [1, 1, 1, 1]
    # store DGE engine per group: alternate HWDGE (sync) / SWDGE (gpsimd,
    # Pool) mid-stream; the tail is all-HWDGE because SWDGE preps serialize
    # ~1us each on the Pool engine, which sits on the tail critical path.
    YENG = ["sync" if (gi % 2 == 0 or gi >= len(YGRP) - 3) else "gpsimd"
            for gi in range(len(YGRP))]

    with tile.TileContext(nc) as tc:
        with ExitStack() as ctx:
            const = ctx.enter_context(tc.tile_pool(name="const", bufs=1))
            xupool = ctx.enter_context(tc.tile_pool(name="xupool", bufs=5))
            zpool = ctx.enter_context(tc.tile_pool(name="zpool", bufs=1))
            ypool = ctx.enter_context(tc.tile_pool(name="ypool", bufs=6))
            psumA = ctx.enter_context(
                tc.tile_pool(name="psumA", bufs=1, space="PSUM"))
            psumB = ctx.enter_context(
                tc.tile_pool(name="psumB", bufs=6, space="PSUM"))

            lam_sb = const.tile([P, RT], F32)
            bias_sb = const.tile([P, NT2], F32)
            u2_sb = const.tile([P, RT, OUT], F16)
            psA = [psumA.tile([P, BSH], F32, name=f"psA{m}") for m in range(RT)]

            # --- stage A: z[r, b] = sum_k U1[k, r] * x[b, k]
            def load_u2(ci):
                sl = slice(ci * UCSZ, (ci + 1) * UCSZ)
                nc.sync.dma_start(u2_sb[:, :, sl], U2T_r[:, :, sl])

            k0 = 0
            for ck, clen in enumerate(CHUNKS):
                xu = xupool.tile([P, CMAX, XC], F16, tag="xu", name=f"xu_{ck}")
                nc.sync.dma_start(xu[:, :clen], XU_r[:, k0:k0 + clen, :])
                if ck == 0:
                    # u2 chunks that must ride early (default none: U2T is
                    # needed later than x, so it loads after the XU stream)
                    for ci in range(N_U2_EARLY):
                        load_u2(ci)
                for t in range(clen):
                    k = k0 + t
                    for m in range(RT):
                        nc.tensor.matmul(
                            psA[m][:],
                            xu[:, t, BSH + m * P:BSH + (m + 1) * P],
                            xu[:, t, 0:BSH],
                            start=(k == 0), stop=(k == KT - 1),
                        )
                k0 += clen
            # lam/bias + U2T queue right behind the XU stream (lam/bias are
            # first: z eviction needs them ~1.5us later); each u2 chunk
            # lands while PE works through earlier stage-B n-tiles.
            nc.sync.dma_start(lam_sb[:], lamT)
            nc.sync.dma_start(bias_sb[:], biasT)
            for ci in range(N_U2_EARLY, OUT // UCSZ):
                load_u2(ci)

            # --- z eviction: z = psA * lam, fp16, split DVE / Activation
            z_sb = zpool.tile([P, RT, BSH], F16, tag="z")
            nc.vector.tensor_tensor(
                z_sb[:, 0], psA[0][:],
                lam_sb[:, 0:1].to_broadcast((P, BSH)), mybir.AluOpType.mult)
            nc.scalar.activation(
                z_sb[:, 1], psA[1][:],
                mybir.ActivationFunctionType.Identity,
                scale=lam_sb[:, 1:2])

            # --- stage B: yT[n, b] = sum_r U2T[r, n] z[r, b] + bias[n]
            evict = [
                lambda o, ps, nt: nc.vector.tensor_tensor(
                    o, ps, bias_sb[:, nt:nt + 1].to_broadcast((P, BSH)),
                    mybir.AluOpType.add),
                lambda o, ps, nt: nc.scalar.activation(
                    o, ps, mybir.ActivationFunctionType.Identity,
                    bias=bias_sb[:, nt:nt + 1]),
            ]  # GPSIMD/Pool cannot access PSUM (BIR verifier), so 2-way only
            nt = 0
            for gi, glen in enumerate(YGRP):
                yg = ypool.tile([P, glen, BSH], F16, tag="y", name=f"y_{gi}")
                for j in range(glen):
                    ps = psumB.tile([P, BSH], F32, tag="psB")
                    for m in range(RT):
                        nc.tensor.matmul(
                            ps[:],
                            u2_sb[:, m, nt * P:(nt + 1) * P],
                            z_sb[:, m, :],
                            start=(m == 0), stop=(m == RT - 1),
                        )
                    evict[nt % 2](yg[:, j, :], ps[:], nt)
                    nt += 1
                eng = nc.sync if YENG[gi] == "sync" else nc.gpsimd
                eng.dma_start(yT_r[:, nt - glen:nt, :], yg[:, :glen])

    nc.compile()
    return nc


def build_kernel(mode=None):
    mode = mode or MODE
    nc = bacc.Bacc(
        "TRN2", target_bir_lowering=False, debug=False, enable_asserts=False
    )
    mm_dt = {"fp32": F32, "fp32r": F32R, "bf16x3": BF16, "fp32rc": F32R,
             "fp16": F16}[mode]
    nsplit = 2 if mode in ("bf16x3", "fp32rc") else 1
    # fp16: single-pass half-precision matmuls (1 cyc/row, same PE rate as
    # f32r) with 2-byte DMA streams — halves HBM traffic, which is the
    # bottleneck once the 3-pass compensation is gone. y is stored as fp16
    # too (host upcasts); total DMA 24MB -> 12MB per core. Error ~5e-4.
    y_dt = F16 if mode == "fp16" else F32
    # fp32rc: all inputs arrive as single fp32 streams (no DMA inflation);
    # hi/lo Dekker splits into f32r happen on-device, with the copy/sub
    # passes balanced across POOL and DVE.
    x_dev_split = mode == "fp32rc"
    x_streams = 2 if (nsplit == 2 and not x_dev_split) else 1
    w_streams = 1 if x_dev_split else nsplit
    x_dt = F32 if x_dev_split else mm_dt
    w_dt = F32 if x_dev_split else mm_dt

    if x_streams == 1:
        xTs = [nc.dram_tensor("xT", [IN, BSH], x_dt, kind="ExternalInput").ap()]
    else:
        xTs = [nc.dram_tensor(f"xT_{sfx}", [IN, BSH], mm_dt,
                              kind="ExternalInput").ap() for sfx in ("hi", "lo")]
    if w_streams == 1:
        U1s = [nc.dram_tensor("U1", [IN, R], w_dt, kind="ExternalInput").ap()]
        U2Ts = [nc.dram_tensor("U2T", [R, OUT], w_dt, kind="ExternalInput").ap()]
    else:
        U1s = [nc.dram_tensor(f"U1_{sfx}", [IN, R], mm_dt,
                              kind="ExternalInput").ap() for sfx in ("hi", "lo")]
        U2Ts = [nc.dram_tensor(f"U2T_{sfx}", [R, OUT], mm_dt,
                               kind="ExternalInput").ap() for sfx in ("hi", "lo")]
    lam = nc.dram_tensor("lam", [R], F32, kind="ExternalInput").ap()
    bias = nc.dram_tensor("bias", [OUT], F32, kind="ExternalInput").ap()
    y = nc.dram_tensor("y", [BSH, OUT], y_dt, kind="ExternalOutput").ap()

    with tile.TileContext(nc) as tc:
        with ExitStack() as ctx:
            const = ctx.enter_context(tc.tile_pool(name="const", bufs=1))
            raw_bufs = 2 if x_dev_split else 6
            xpool = ctx.enter_context(tc.tile_pool(name="xpool", bufs=raw_bufs))
            wpool = ctx.enter_context(tc.tile_pool(name="wpool", bufs=raw_bufs))
            spool = ctx.enter_context(
                tc.tile_pool(name="spool", bufs=3))
            u2rpool = ctx.enter_context(tc.tile_pool(name="u2rpool", bufs=2))
            zpool = ctx.enter_context(tc.tile_pool(name="zpool", bufs=1))
            ypool = ctx.enter_context(
                tc.tile_pool(name="ypool", bufs=3 if x_dev_split else 4))
            psumA = ctx.enter_context(tc.tile_pool(name="psumA", bufs=1, space="PSUM"))
            psumB = ctx.enter_context(tc.tile_pool(name="psumB", bufs=4, space="PSUM"))

            # --- stage A: z[r, b] = sum_k U1[k, r] xT[k, b]  (K = IN = 4096)
            # DMAs are batched into multi-k-tile chunks: descriptor-gen cost
            # (~625ns/DMA on the shared HWDGE) is per dma_start, so fewer +
            # bigger transfers keep the DMA pipeline at bandwidth. The first
            # chunk is small so PE starts early. Constant loads (u2, bias,
            # lam) are sprinkled in so they fill otherwise-idle DMA time.
            CHUNKS = [1, 3] + [4] * 7  # k-tiles per DMA chunk, sum = KT
            CMAX = max(CHUNKS)
            psA = [psumA.tile([P, BSH], F32, name=f"psA{m}") for m in range(RT)]
            lam_sb = const.tile([P, RT], F32)
            bias_bc = const.tile([P, OUT], F32)
            u2_sb = [const.tile([P, RT, OUT], mm_dt, tag=f"u2{s}", name=f"u2{s}")
                     for s in range(nsplit)]
            U2T_r = [u.rearrange("(kt p) n -> p kt n", p=P) for u in U2Ts]
            # u2 load chunk width. For fp32rc most u2 split work is deferred
            # into stage B (where DVE is otherwise idle), chunk = one n-tile.
            csz = 512 if x_dev_split else 1024
            u2_jobs = [(s, ci) for s in range(w_streams)
                       for ci in range(OUT // csz)]
            # fp16 is DMA-bus-bound end to end: every byte of U2T moved
            # during stage A delays x (and thus stage A's critical path) by
            # the same bus time, so defer ALL u2 loads to stage B.
            n_u2_stage_a = (3 if x_dev_split else
                            (0 if mode == "fp16" else len(u2_jobs)))

            def load_u2_chunk(s, ci):
                sl = slice(ci * csz, (ci + 1) * csz)
                if not x_dev_split:
                    nc.sync.dma_start(u2_sb[s][:, :, sl], U2T_r[s][:, :, sl])
                    return
                raw = u2rpool.tile([P, RT, csz], F32, tag="u2raw",
                                   name=f"u2raw_{ci}")
                nc.sync.dma_start(raw[:], U2T_r[0][:, :, sl])
                nc.vector.tensor_copy(u2_sb[0][:, :, sl], raw[:])
                nc.vector.tensor_tensor(
                    u2_sb[1][:, :, sl], raw[:], u2_sb[0][:, :, sl],
                    mybir.AluOpType.subtract,
                )

            k0 = 0
            for ck, clen in enumerate(CHUNKS):
                xts, u1ts = [], []
                for s in range(x_streams):
                    xt = xpool.tile([P, CMAX, BSH], x_dt, tag=f"xt{s}",
                                    name=f"xt{s}_{ck}")
                    nc.sync.dma_start(
                        xt[:, :clen],
                        xTs[s][k0 * P:(k0 + clen) * P, :]
                        .rearrange("(t p) b -> p t b", p=P),
                    )
                    xts.append(xt)
                for s in range(w_streams):
                    u1t = wpool.tile([P, CMAX, R], w_dt, tag=f"u1{s}",
                                     name=f"u1{s}_{ck}")
                    nc.sync.dma_start(
                        u1t[:, :clen],
                        U1s[s][k0 * P:(k0 + clen) * P, :]
                        .rearrange("(t p) r -> p t r", p=P),
                    )
                    u1ts.append(u1t)
                if x_dev_split:
                    # Dekker split on device (DVE): hi = rnd_f32r(x),
                    # lo = rnd_f32r(x - hi); chunk-granular ops.
                    xh = spool.tile([P, CMAX, BSH], F32R, tag="xh",
                                    name=f"xh_{ck}")
                    xl = spool.tile([P, CMAX, BSH], F32R, tag="xl",
                                    name=f"xl_{ck}")
                    u1h = u2rpool.tile([P, CMAX, R], F32R, tag="u1h",
                                       name=f"u1h_{ck}")
                    u1l = u2rpool.tile([P, CMAX, R], F32R, tag="u1l",
                                       name=f"u1l_{ck}")
                    nc.vector.tensor_copy(xh[:, :clen], xts[0][:, :clen])
                    nc.vector.tensor_tensor(
                        xl[:, :clen], xts[0][:, :clen], xh[:, :clen],
                        mybir.AluOpType.subtract,
                    )
                    nc.vector.tensor_copy(u1h[:, :clen], u1ts[0][:, :clen])
                    nc.vector.tensor_tensor(
                        u1l[:, :clen], u1ts[0][:, :clen], u1h[:, :clen],
                        mybir.AluOpType.subtract,
                    )
                    xts = [xh, xl]
                    u1ts = [u1h, u1l]
                if ck == 1:
                    nc.sync.dma_start(lam_sb[:], lam.rearrange("(o p) -> p o", p=P))
                    nc.sync.dma_start(bias_bc[0:1, :],
                                      bias.rearrange("(a n) -> a n", a=1))
                if ck == (len(CHUNKS) - 1 if x_dev_split else 2):
                    nc.gpsimd.partition_broadcast(bias_bc[:], bias_bc[0:1, :])
                if ck >= 3 and u2_jobs and (
                        (OUT // csz) * w_streams - len(u2_jobs) < n_u2_stage_a):
                    load_u2_chunk(*u2_jobs.pop(0))
                    if ck == len(CHUNKS) - 1 and not x_dev_split:
                        while u2_jobs:  # flush any leftovers (bf16x3)
                            load_u2_chunk(*u2_jobs.pop(0))
                for t in range(clen):
                    k = k0 + t
                    for m in range(RT):
                        _stage_a_matmuls(
                            nc, psA[m][:],
                            [u[:, t, m * P:(m + 1) * P] for u in u1ts],
                            [x[:, t, :] for x in xts],
                            k, KT - 1,
                        )
                k0 += clen

            # --- z eviction: z = psA * lam  (+ hi/lo split on device for bf16x3)
            z_sb = []
            if nsplit == 1:
                zt = zpool.tile([P, RT, BSH], mm_dt, tag="z")
                for m in range(RT):
                    nc.vector.tensor_tensor(
                        zt[:, m], psA[m][:],
                        lam_sb[:, m:m + 1].to_broadcast((P, BSH)),
                        mybir.AluOpType.mult,
                    )
                z_sb = [zt]
            else:
                lo_dt = F32R if x_dev_split else BF16
                zf = zpool.tile([P, RT, BSH], F32, tag="zf")
                z_hi = zpool.tile([P, RT, BSH], lo_dt, tag="zhi")
                z_lo = zpool.tile([P, RT, BSH], lo_dt, tag="zlo")
                for m in range(RT):
                    nc.vector.tensor_tensor(
                        zf[:, m], psA[m][:],
                        lam_sb[:, m:m + 1].to_broadcast((P, BSH)),
                        mybir.AluOpType.mult,
                    )
                    nc.vector.tensor_copy(z_hi[:, m], zf[:, m])
                    nc.vector.tensor_tensor(
                        z_lo[:, m], zf[:, m], z_hi[:, m],
                        mybir.AluOpType.subtract,
                    )
                z_sb = [z_hi, z_lo]

            # --- stage B: y[b, n] = sum_r z[r, b] U2T[r, n] + bias[n]
            # y goes out in [P, 2048] chunks via gpsimd (SWDGE) so store
            # descriptor-gen doesn't contend with the HWDGE load pipeline.
            combos = [(0, 0)] if nsplit == 1 else [(0, 0), (0, 1), (1, 0)]
            YC = 1024  # columns per output store
            for mb in range(MB):
                y_sb = None
                for n in range(NT):
                    # fp32rc: remaining u2 chunks split just-in-time here —
                    # chunk n+2 is produced while tiles (mb=0, n) compute,
                    # keeping the big u2 split off stage A's critical DVE path.
                    if u2_jobs:
                        load_u2_chunk(*u2_jobs.pop(0))
                    ps = psumB.tile([P, 512], F32, tag="psB")
                    for kr in range(RT):
                        for ci, (li, ri) in enumerate(combos):
                            nc.tensor.matmul(
                                ps[:],
                                z_sb[li][:, kr, mb * P:(mb + 1) * P],
                                u2_sb[ri][:, kr, n * 512:(n + 1) * 512],
                                start=(kr == 0 and ci == 0),
                                stop=(kr == RT - 1 and ci == len(combos) - 1),
                            )
                    # last row-block streams out per 512-col tile so the
                    # kernel tail is one small store, not a 1MB one
                    yc = 512 if (mb == MB - 1 and mm_dt == F32) else YC
                    if n % (yc // 512) == 0:
                        y_sb = ypool.tile([P, yc], y_dt, tag="y", name=f"y_{mb}_{n}")
                    off = (n % (yc // 512)) * 512
                    nc.vector.tensor_tensor(
                        y_sb[:, off:off + 512], ps[:],
                        bias_bc[:, n * 512:(n + 1) * 512],
                        mybir.AluOpType.add,
                    )
                    if (n + 1) % (yc // 512) == 0:
                        nc.gpsimd.dma_start(
                            y[mb * P:(mb + 1) * P,
                              (n + 1) * 512 - yc:(n + 1) * 512],
                            y_sb[:, :yc],
                        )

    nc.compile()
    return nc


def _round_f32r(a):
    """Round fp32 -> tf32-like (11 mantissa bits), round-half-to-even."""
    bits = a.view(np.uint32)
    rounded = (bits.astype(np.uint64) + 0x7FF + ((bits >> 12) & 1)) & 0xFFFFF000
    return rounded.astype(np.uint32).view(np.float32)


def _split_bf16(a):
    import ml_dtypes
    hi = a.astype(ml_dtypes.bfloat16)
    lo = (a - hi.astype(np.float32)).astype(ml_dtypes.bfloat16)
    return hi, lo


def prep_in_maps(x, U1, U2, lam, bias, mode=None):
    mode = mode or MODE
    x = np.ascontiguousarray(np.asarray(x, dtype=np.float32))
    U1 = np.ascontiguousarray(np.asarray(U1, dtype=np.float32))
    U2T = np.ascontiguousarray(np.asarray(U2, dtype=np.float32).T)
    lam = np.ascontiguousarray(np.asarray(lam, dtype=np.float32))
    bias = np.ascontiguousarray(np.asarray(bias, dtype=np.float32))

    in_maps = []
    for i in range(NCORES):
        xT_i = np.ascontiguousarray(x[i * BSH:(i + 1) * BSH, :].T)
        if mode == "fp32":
            m = {"xT": xT_i, "U1": U1, "U2T": U2T}
        elif mode == "fp32rc":
            m = {"xT": xT_i, "U1": U1, "U2T": U2T}
        elif mode == "fp16":
            m = {"XU": np.concatenate([xT_i, U1], axis=1).astype(np.float16),
                 "U2T": U2T.astype(np.float16),
                 "lamT": np.ascontiguousarray(lam.reshape(R // P, P).T),
                 "biasT": np.ascontiguousarray(bias.reshape(OUT // P, P).T)}
            in_maps.append(m)
            continue
        elif mode == "fp32r":
            m = {"xT": _round_f32r(xT_i), "U1": _round_f32r(U1),
                 "U2T": _round_f32r(U2T)}
        else:
            xh, xl = _split_bf16(xT_i)
            u1h, u1l = _split_bf16(U1)
            u2h, u2l = _split_bf16(U2T)
            m = {"xT_hi": xh, "xT_lo": xl, "U1_hi": u1h, "U1_lo": u1l,
                 "U2T_hi": u2h, "U2T_lo": u2l}
        m["lam"] = lam
        m["bias"] = bias
        in_maps.append(m)
    return in_maps


_NC_CACHE = {}


def _build_runner(nc):
    """PJRT runner WITHOUT output-buffer donation.

    The stock run_bass_via_pjrt path donates zero-initialized output
    buffers into the bass_exec custom call; on this axon stack that
    donation intermittently corrupted outputs or crashed the exec unit
    (~1 in 5 fresh-process runs for larger-input kernels). With donation
    off (fresh result buffers, 23/23 clean trials), execution is
    deterministic. Our kernel writes every output element, so the zero
    initial contents are irrelevant.
    """
    import jax
    from jax.sharding import Mesh, PartitionSpec, NamedSharding
    from jax.experimental.shard_map import shard_map
    from concourse import bass2jax

    bass2jax.install_neuronx_cc_hook()
    partition_name = nc.partition_id_tensor.name if nc.partition_id_tensor else None
    in_names, out_names, out_avals, zero_outs = [], [], [], []
    for alloc in nc.m.functions[0].allocations:
        if not isinstance(alloc, mybir.MemoryLocationSet):
            continue
        name = alloc.memorylocations[0].name
        if alloc.kind == "ExternalInput":
            if name != partition_name:
                in_names.append(name)
        elif alloc.kind == "ExternalOutput":
            out_names.append(name)
            shape = tuple(alloc.tensor_shape)
            dtype = mybir.dt.np(alloc.dtype)
            out_avals.append(jax.core.ShapedArray(shape, dtype))
            zero_outs.append(np.zeros(shape, dtype))
    all_in_names = list(in_names) + list(out_names)
    if partition_name is not None:
        all_in_names.append(partition_name)

    def _body(*args):
        operands = list(args)
        if partition_name is not None:
            operands.append(bass2jax.partition_id_tensor())
        return tuple(bass2jax._bass_exec_p.bind(
            *operands,
            out_avals=tuple(out_avals),
            in_names=tuple(all_in_names),
            out_names=tuple(out_names),
            lowering_input_output_aliases=(),
            sim_require_finite=True,
            sim_require_nnan=True,
            nc=nc,
        ))

    devices = jax.devices()[:NCORES]
    mesh = Mesh(np.asarray(devices), ("core",))
    nin = len(in_names) + len(zero_outs)
    fn = jax.jit(
        shard_map(_body, mesh=mesh,
                  in_specs=(PartitionSpec("core"),) * nin,
                  out_specs=(PartitionSpec("core"),) * len(out_names),
                  check_rep=False),
        keep_unused=True,
    )
    sharding = NamedSharding(mesh, PartitionSpec("core"))
    dev_zeros = [
        jax.device_put(
            np.zeros((NCORES * z.shape[0], *z.shape[1:]), z.dtype), sharding)
        for z in zero_outs
    ]

    def run(in_maps):
        concat_in = [
            jax.device_put(
                np.concatenate([np.asarray(in_maps[c][nm])
                                for c in range(NCORES)], axis=0), sharding)
            for nm in in_names
        ]
        outs = fn(*concat_in, *dev_zeros)
        return np.asarray(outs[0])  # (NCORES*BSH, OUT) in batch order

    return run


_BASS_CACHE = {}


def _run_once(mode, x, U1, U2, lam, bias, _trace, _tmpdir):
    if mode not in _NC_CACHE:
        nc = build_kernel_fp16() if mode == "fp16" else build_kernel(mode)
        _BASS_CACHE[mode] = nc
        _NC_CACHE[mode] = _build_runner(nc)
    in_maps = prep_in_maps(x, U1, U2, lam, bias, mode)
    out = _NC_CACHE[mode](in_maps)
    if mode == "fp16":
        # runner concatenates per-core yT [OUT, BSH] along axis 0; core i's
        # yT.T is y rows [i*BSH:(i+1)*BSH]
        out = np.ascontiguousarray(
            out.reshape(NCORES, OUT, BSH).transpose(0, 2, 1).reshape(B, OUT))
    if out.dtype != np.float32:
        out = out.astype(np.float32)
    return out


def kernel(x, U1, U2, lam, bias, _trace=False, _tmpdir=None, _mode=None):
    # Device execution through the axon tunnel can very occasionally fail
    # transiently (NRT_EXEC_UNIT_UNRECOVERABLE); retry the same mode, then
    # fall back to the plain-fp32 kernel before giving up.
    mode = _mode or MODE
    attempts = [mode, mode, "fp32", "fp32"]
    last_err = None
    for i, m in enumerate(attempts):
        try:
            return _run_once(m, x, U1, U2, lam, bias, _trace, _tmpdir)
        except Exception as e:  # noqa: BLE001 - deliberate retry barrier
            last_err = e
            import time as _time
            _time.sleep(2.0 * (i + 1))
    raise last_err

